# revision 37
# baseline (speedup 1.0000x reference)
"""Trainium2 Bass kernel for a dense transformer block.

Block: x = LN1(x + MHA(x)); x = LN2(x + FFN(x))
Shapes: B=2, T=2048, C=1024, H=16, DK=64, FF=4096, fp32 in/out, bf16
internally.

Sharding: token-parallel over 8 cores (core r: batch r//4, query chunk
c = r%4 of 512 tokens, all 16 heads) with cross-core K/V sharing: each
core computes K/V for only its OWN 512 tokens and broadcasts them to
its 3 quad peers over D2D remote DMA (XOR-relative dests, so the
program is uniform SPMD). Peer slot d-1 holds chunk (c ^ d); per-core
visibility (causal masking of whole peer chunks) is data: a 0/1
indicator multiplied into the V tiles and the softmax-denominator
column. The own chunk's diagonal tiles are processed from the local
K/V buffers with static triangle masks. Sync: kernel-entry barrier,
then sends -> DMA drain -> quad AllGather barrier (then_inc) -> reader
sem waits (inserted post-tile-scheduling, since the schedulers don't
model RDMA semaphores). Activations stay feature-major so every linear
is matmul(lhsT=W_tile, rhs=X^T) with a 512-wide moving dim, and all
per-feature affine ops are per-partition scalars.
"""

import os
import math
import numpy as np
import ml_dtypes

import concourse.bass as bass
import concourse.bass_isa as bass_isa
import concourse.mybir as mybir
import concourse.tile as tile
from concourse import bacc
from concourse.bass_utils import run_bass_kernel_spmd

BF16_NP = ml_dtypes.bfloat16

F32 = mybir.dt.float32
F32R = mybir.dt.float32r
BF16 = mybir.dt.bfloat16
AF = mybir.ActivationFunctionType
ALU = mybir.AluOpType

B, T, C = 2, 2048, 1024
H, DK = 16, 64
FF = 4 * C
EPS = 1e-5
NCORES = 8
QCH = 512            # query tokens per core (= own kv chunk)
CT = C // 128        # 8 c-tiles
FFT = FF // 128      # 32 ff-tiles
SCALE = 1.0 / math.sqrt(DK)
EBASE = math.exp(SCALE)   # exp(x*SCALE) == EBASE ** x (gpsimd pow path)
RG = [[0, 1, 2, 3], [4, 5, 6, 7]]

_CACHE = {}


def _ln_stats_step(nc, ps_pool, sb_pool, m_ps, sq_ps, z_ap, ones_col, c,
                   sq_eng=None):
    """Accumulate per-token sum and sum-of-squares for one c-tile."""
    nc.tensor.matmul(m_ps[:, :], ones_col[:, :], z_ap,
                     start=(c == 0), stop=(c == CT - 1))
    zsq = sb_pool.tile([128, 512], BF16, tag="ln_zsq")
    (sq_eng or nc.gpsimd).tensor_mul(zsq[:, :], z_ap, z_ap)
    nc.tensor.matmul(sq_ps[:, :], ones_col[:, :], zsq[:, :],
                     start=(c == 0), stop=(c == CT - 1))


def _layernorm_feature_major(nc, tc, persist, ps_pool, sb_pool, z_tiles, y_tile,
                             gamma_sb, beta_sb, ones_col, ones_row, eps_t,
                             out_dma=None, stats=None):
    """y = LN(z) over the feature axis (partitions x 8 c-tiles).

    z_tiles: callable c -> AP [128, 512] (bf16), y_tile: [128, 8, 512].
    gamma_sb/beta_sb: [128, 8] fp32. Stats per token via ones-matmuls
    (or already accumulated in `stats`=(m_ps, sq_ps)).
    """
    if stats is not None:
        m_ps, sq_ps = stats
    else:
        m_ps = ps_pool.tile([1, 512], F32, tag="ln_m")
        sq_ps = ps_pool.tile([1, 512], F32, tag="ln_sq")
        for c in range(CT):
            _ln_stats_step(nc, ps_pool, sb_pool, m_ps, sq_ps, z_tiles(c),
                           ones_col, c)
    mean_sb = sb_pool.tile([1, 512], F32R, tag="ln_mean")
    nc.vector.tensor_scalar(out=mean_sb[:, :], in0=m_ps[:, :],
                            scalar1=1.0 / C, scalar2=0.0,
                            op0=ALU.mult, op1=ALU.add)
    msq_sb = sb_pool.tile([1, 512], F32, tag="ln_msq")
    nc.vector.tensor_scalar(out=msq_sb[:, :], in0=sq_ps[:, :],
                            scalar1=1.0 / C, scalar2=0.0,
                            op0=ALU.mult, op1=ALU.add)
    var_sb = sb_pool.tile([1, 512], F32, tag="ln_var")
    nc.vector.tensor_mul(var_sb[:, :], mean_sb[:, :], mean_sb[:, :])
    nc.vector.tensor_sub(var_sb[:, :], msq_sb[:, :], var_sb[:, :])
    sd_sb = sb_pool.tile([1, 512], F32, tag="ln_sd")
    nc.scalar.activation(sd_sb[:, :], var_sb[:, :], AF.Sqrt, bias=eps_t[:, :])
    rstd_sb = sb_pool.tile([1, 512], F32R, tag="ln_rstd")
    nc.vector.reciprocal(rstd_sb[:, :], sd_sb[:, :])

    bcm_ps = ps_pool.tile([128, 512], F32, tag="ln_bcm")
    nc.tensor.matmul(bcm_ps[:, :], ones_row[0:1, :], mean_sb[:, :],
                     start=True, stop=True)
    bcr_ps = ps_pool.tile([128, 512], F32, tag="ln_bcr")
    nc.tensor.matmul(bcr_ps[:, :], ones_row[0:1, :], rstd_sb[:, :],
                     start=True, stop=True)
    # bf16 broadcasts: all-bf16 DVE ops below run in 4x mode; the ~0.4%
    # rounding on mean/rstd is well inside the error budget.
    bcm_sb = sb_pool.tile([128, 512], BF16, tag="ln_bcm_sb")
    nc.vector.tensor_copy(bcm_sb[:, :], bcm_ps[:, :])
    bcr_sb = sb_pool.tile([128, 512], BF16, tag="ln_bcr_sb")
    nc.vector.tensor_copy(bcr_sb[:, :], bcr_ps[:, :])

    act_affine = y_tile.tensor.dtype != BF16
    for c in range(CT):
        t0 = sb_pool.tile([128, 512], BF16, tag="ln_t0")
        nc.vector.tensor_sub(t0[:, :], z_tiles(c), bcm_sb[:, :])
        nc.vector.tensor_mul(t0[:, :], t0[:, :], bcr_sb[:, :])
        if act_affine:
            # f32 y (the kernel output): affine on the otherwise-idle
            # Act engine; out = Identity(t0*gamma + beta)
            nc.scalar.activation(
                y_tile[:, c, :], t0[:, :], AF.Identity,
                bias=beta_sb[:, c:c + 1], scale=gamma_sb[:, c:c + 1])
        else:
            nc.vector.tensor_scalar(
                out=y_tile[:, c, :], in0=t0[:, :],
                scalar1=gamma_sb[:, c:c + 1], scalar2=beta_sb[:, c:c + 1],
                op0=ALU.mult, op1=ALU.add)
        if out_dma is not None:
            out_dma(c)
    return bcm_sb, bcr_sb


def _build():
    nc = bacc.Bacc("TRN2", target_bir_lowering=False, debug=False,
                   num_devices=NCORES)

    xc_d = nc.dram_tensor("xc", [C, QCH], BF16, kind="ExternalInput")
    wq = nc.dram_tensor("wq", [C, C], BF16, kind="ExternalInput")
    wk = nc.dram_tensor("wk", [C, C], BF16, kind="ExternalInput")
    wv = nc.dram_tensor("wv", [C, C], BF16, kind="ExternalInput")
    wo = nc.dram_tensor("wo", [8, 128, C], BF16, kind="ExternalInput")
    w1 = nc.dram_tensor("w1", [C, FF], BF16, kind="ExternalInput")
    w2 = nc.dram_tensor("w2", [FF, C], BF16, kind="ExternalInput")
    # masks[0:4]: causal triangles; masks[4][:, 0:128]: identity (PE transp.)
    masks = nc.dram_tensor("masks", [5, 128, 512], BF16, kind="ExternalInput")
    kvind = nc.dram_tensor("kvind", [12, 128, 8], F32, kind="ExternalInput")
    scal = nc.dram_tensor("scal", [128, 176], F32, kind="ExternalInput")
    out = nc.dram_tensor("out", [C, QCH], F32, kind="ExternalOutput")
    dbg = None
    if os.environ.get("KERNEL_DEBUG", "0") == "1":
        dbg = nc.dram_tensor("dbg", [128, 8, 512], BF16,
                             kind="ExternalOutput")

    arrk_sem = nc.alloc_semaphore("k_arrived")
    arrv_sem = nc.alloc_semaphore("v_arrived")
    rsem = nc.alloc_semaphore("rdma_rsem")
    lsem = nc.alloc_semaphore("rdma_lsem")



    def bcast4(out_ap, in_ap, d):
        """remote_dma_broadcast with a 4-slot dest list (1 real XOR dest).

        Same ucode contract as the stock helper (power-of-2 n_dests, the
        RMTV ^2 lane-balance stays in range, no D2D slots needed for
        intra-device transfers) but with 4 lane-slots instead of 8.
        """
        free_b = in_ap.free_size() * mybir.dt.size(in_ap.dtype)
        packed = [-1] * 8
        packed[d] = d  # (rid=0) << 3 | tpb=d
        inst = nc.gpsimd.add_instruction(
            bass_isa.InstRemoteDMABroadcastDescs(
                name=f"I-{nc.next_id()}",
                ins=[nc.gpsimd.lower_ap(in_ap, for_isa=True)],
                outs=[nc.gpsimd.lower_ap(out_ap, for_isa=True)],
                free_dim_bytes=free_b,
                remote_sem=rsem.num,
                remote_sem_name=rsem.name,
                local_sem_update=bass.create_sync_update(lsem, 16),
                queue_num=0,
                dests=packed,
                relative=True,
            ))
        return nc.gpsimd._track_prepare_only(inst, 0)

    trig_k = trig_v = None
    k_preps = []
    slot_pe_readers = []   # PE matmuls reading krecv
    slot_dve_readers = []  # DVE ops reading vrecv

    with tile.TileContext(nc) as tc, nc.allow_low_precision(
            reason="bf16 tiles feed matmuls; fp32 accumulation in PSUM"):
        with (
            tc.tile_pool(name="persist", bufs=1) as persist,
            tc.tile_pool(name="post", bufs=1) as post,
            tc.tile_pool(name="w1pre", bufs=1) as w1pre_pool,
        ):
            # Constants / small inputs
            ones_f32 = persist.tile([128, 128], F32)
            nc.vector.memset(ones_f32[:, :], 1.0)
            ones_col = persist.tile([128, 1], BF16)
            nc.vector.tensor_copy(ones_col[:, :], ones_f32[:, 0:1])
            ones_bf = persist.tile([128, 8], BF16)
            nc.vector.tensor_copy(ones_bf[:, :], ones_f32[:, 0:8])
            ones_row = persist.tile([1, 128], F32R)
            nc.vector.tensor_copy(ones_row[:, :], ones_f32[0:1, :])
            eps_t = persist.tile([1, 1], F32)
            nc.vector.memset(eps_t[:, :], EPS)
            ebase = persist.tile([128, 512], F32)
            nc.vector.memset(ebase[:, :], EBASE)

            scal_sb = persist.tile([128, 176], F32)
            bq_sb = scal_sb[:, 0:8]
            bk_sb = scal_sb[:, 8:16]
            bv_sb = scal_sb[0:64, 16:32]
            bo_sb = scal_sb[:, 32:40]
            b1_sb = scal_sb[:, 40:72]
            b2_sb = scal_sb[:, 72:80]
            g1_sb = scal_sb[:, 80:88]
            bt1_sb = scal_sb[:, 88:96]
            g2_sb = scal_sb[:, 96:104]
            bt2_sb = scal_sb[:, 104:112]
            uneg_sb = scal_sb[:, 112:144]
            vb1_sb = scal_sb[:, 144:176]
            kvind_sb = persist.tile([128, 12, 8], F32)
            nc.gpsimd.dma_start(out=kvind_sb[:, :, :],
                                in_=kvind.rearrange("j p c -> p j c"))

            z1 = post.tile([128, 8, 512], BF16, tag="z")
            y1 = post.tile([128, 8, 512], BF16, tag="y")

            with (
                tc.tile_pool(name="span1", bufs=1) as span1,
                tc.tile_pool(name="kvbuf", bufs=1) as kvbuf,
                tc.tile_pool(name="wo_sb", bufs=2) as wo_pool,
            ):
                # Own-chunk x^T (feature-major), also the residual input.
                xq = span1.tile([128, 8, 512], BF16)
                for ci in range(CT):
                    nc.scalar.dma_start(
                        out=xq[:, ci, :],
                        in_=xc_d[128 * ci:128 * ci + 128, :])
                nc.gpsimd.dma_start(out=scal_sb[:, :], in_=scal[:, :])
                masks_sb = span1.tile([128, 5, 512], BF16)

                # K/V own + recv buffers (alive for all of attention)
                ks = kvbuf.tile([128, 8, 512], BF16)
                vs = kvbuf.tile([128, 4, 1024], BF16)
                krecv = kvbuf.tile([128, 3, 8, 512], BF16)
                vrecv = kvbuf.tile([128, 3, 4, 1024], BF16)

                # ---- K own: 8 feature slabs (head pairs) ----
                with (
                    tc.tile_pool(name="wkv", bufs=3) as wkv,
                    tc.tile_pool(name="kv_ps", bufs=2, space="PSUM") as kv_ps,
                ):
                    for s in range(8):
                        wks = wkv.tile([128, 8, 128], BF16, tag="wks")
                        nc.sync.dma_start(
                            out=wks[:, :, :],
                            in_=wk[:, 128 * s:128 * s + 128]
                            .rearrange("(a p) f -> p a f", p=128))
                        kps = kv_ps.tile([128, 512], F32, tag="kvp")
                        for ci in range(CT):
                            nc.tensor.matmul(
                                kps[:, :], wks[:, ci, :], xq[:, ci, :],
                                start=(ci == 0), stop=(ci == CT - 1))
                        nc.vector.tensor_scalar_add(
                            out=ks[:, s, :], in0=kps[:, :],
                            scalar1=bk_sb[:, s:s + 1])
                    # K broadcasts to the 3 quad peers (XOR-relative)
                    for d in (1, 2, 3):
                        k_preps.append(bcast4(krecv[:, d - 1, :, :],
                                              ks[:, :, :], d))
                    trig_k = nc.gpsimd.trigger_dma(count=None)

                    # ---- V own: 4 token tiles x 4 feature chunks ----
                    for fq in range(4):
                        wvs = wkv.tile([128, 8, 256], BF16, tag="wvs")
                        nc.sync.dma_start(
                            out=wvs[:, :, :],
                            in_=wv[:, 256 * fq:256 * fq + 256]
                            .rearrange("(a p) f -> p a f", p=128))
                        for tt in range(4):
                            vps = kv_ps.tile([128, 256], F32, tag="kvp")
                            for ci in range(CT):
                                nc.tensor.matmul(
                                    vps[:, :],
                                    xq[:, ci, 128 * tt:128 * tt + 128],
                                    wvs[:, ci, :],
                                    start=(ci == 0), stop=(ci == CT - 1))
                            nc.scalar.copy(
                                vs[:, tt, 256 * fq:256 * fq + 256],
                                vps[:, :])
                    for d in (1, 2, 3):
                        bcast4(vrecv[:, d - 1, :, :], vs[:, :, :], d)
                    trig_v = nc.gpsimd.trigger_dma(count=None)

                # MHA output, feature-major: head pair on partitions
                mha = span1.tile([128, 8, 512], BF16)

                # ------------- Attention: 4 passes of 4 heads -------------
                with (
                    tc.tile_pool(name="wq_sb", bufs=1) as wq_pool,
                    tc.tile_pool(name="attn_sb", bufs=4) as attn_sb,
                    tc.tile_pool(name="e_sb", bufs=6) as e_sb,
                    tc.tile_pool(name="stg_sb", bufs=4) as stg_sb,
                    tc.tile_pool(name="n_sb", bufs=4) as n_sb,
                ):
                    # full Wq upfront; all 4 passes' Q projections run
                    # before the peer gate so the kv-exchange barrier hides
                    # under them.
                    for mj in range(5):
                        nc.gpsimd.dma_start(
                            out=masks_sb[:, mj, :],
                            in_=masks[mj, :, :])
                    wq_sb = wq_pool.tile([128, 8, 1024], BF16, tag="wqf")
                    for qq in range(4):
                        nc.sync.dma_start(
                            out=wq_sb[:, 2 * qq:2 * qq + 2, :],
                            in_=wq[256 * qq:256 * qq + 256, :]
                            .rearrange("(a p) f -> p a f", p=128))
                    tc.tile_set_cur_wait(0.032)
                    w1pre = w1pre_pool.tile([128, 8, 512], BF16, name="w1s0")
                    qts = []
                    with tc.tile_pool(name="q_ps", bufs=2,
                                      space="PSUM") as q_ps:
                        for qr in range(4):
                            qt = attn_sb.tile([128, 2, 512], BF16, tag="qt",
                                              name=f"qt{qr}")
                            for kd in range(2):
                                qps = q_ps.tile([128, 512], F32, tag="qp")
                                for ci in range(CT):
                                    nc.tensor.matmul(
                                        qps[:, :],
                                        wq_sb[:, ci,
                                              256 * qr + 128 * kd:
                                              256 * qr + 128 * kd + 128],
                                        xq[:, ci, :],
                                        start=(ci == 0), stop=(ci == CT - 1))
                                nc.vector.tensor_scalar_add(
                                    out=qt[:, kd, :], in0=qps[:, :],
                                    scalar1=bq_sb[:, 2 * qr + kd:
                                                  2 * qr + kd + 1])
                            qts.append(qt)

                    ident = masks_sb[:, 4, 0:128]
                    # greedy build-time load balance of exp tiles between
                    # the Act engine (direct from PSUM) and GpSimd pow
                    # (staged PSUM->SBUF by DMA: GPSIMD cannot read PSUM;
                    # the DMA engines are idle during attention).
                    exp_t = [0.0, 0.0]

                    def emit_exp(e_ap, l_ap, stage_pool, bias_act=None,
                                 bias_pool=None):
                        """Visibility of whole peer kv-tiles is folded into
                        the exp as a large negative per-partition bias
                        (bias_act post-scale for Act, bias_pool pre-scale
                        added during the PSUM->SBUF staging copy)."""
                        if exp_t[0] + 612 <= exp_t[1] + 1167:
                            exp_t[0] += 612
                            nc.scalar.activation(
                                e_ap, l_ap, AF.Exp, scale=SCALE,
                                bias=bias_act if bias_act is not None
                                else 0.0)
                        else:
                            exp_t[1] += 1167
                            stg = stage_pool.tile([128, 512], F32,
                                                  tag="pstg")
                            if bias_pool is None:
                                nc.vector.tensor_copy(stg[:, :], l_ap)
                            else:
                                nc.vector.tensor_scalar(
                                    out=stg[:, :], in0=l_ap,
                                    scalar1=bias_pool, scalar2=None,
                                    op0=ALU.add)
                            nc.gpsimd.tensor_tensor(e_ap, ebase[:, :],
                                                    stg[:, :], ALU.pow)

                    with (
                        tc.tile_pool(name="l_ps", bufs=4,
                                     space="PSUM") as l_ps,
                        tc.tile_pool(name="o_ps", bufs=1,
                                     space="PSUM") as o_ps,
                        tc.tile_pool(name="t_ps", bufs=1,
                                     space="PSUM") as t_ps,
                    ):
                        for qr in range(4):
                            qt = qts[qr]
                            # AV accumulators: head pair x 4 query tiles x
                            # 64 feats (query-major), one PSUM bank each;
                            # softmax denominators accumulate separately in
                            # a shared [128, 16] bank.
                            # One PSUM zero-region (2KB bank) per tile:
                            # exactly ONE start=True per region (first
                            # matmul), first-touch writes of other slots
                            # overwrite via pending-zero, one stop=True on
                            # the region's last matmul.
                            oacc = [o_ps.tile([128, 8, 64], F32,
                                              tag=f"o{hp}",
                                              name=f"o{hp}_{qr}")
                                    for hp in range(2)]
                            dacc_pad = o_ps.tile([128, 512], F32,
                                                 tag="dacc",
                                                 name=f"dacc_{qr}")
                            dacc = dacc_pad[:, 0:16]

                            def _tile_pass(ksrc, vsrc, mask_mj, bias_j,
                                           start, stop, record=False):
                                """One 128-token kv tile: logits+exp+AV.

                                ksrc: (lh, p) -> lhsT AP [64, 128]
                                vsrc: g -> AP [128, 64] (head g V tile)
                                mask_mj: diag mask index or None
                                bias_j: peer-slot index for visibility
                                bias columns, or None (diag, visible)
                                """
                                es = []
                                for p in range(2):
                                    for lh in range(2):
                                        lps = l_ps.tile([128, 512], F32,
                                                        tag="lg")
                                        mm = nc.tensor.matmul(
                                            lps[:, :], ksrc(lh, p),
                                            qt[64 * lh:64 * lh + 64, p, :],
                                            start=True, stop=True,
                                            tile_position=(64 * lh, 0))
                                        if record:
                                            slot_pe_readers.append(mm.ins)
                                        e = e_sb.tile([128, 512], BF16,
                                                      tag="e",
                                                      name=f"e{p}{lh}")
                                        if bias_j is None:
                                            emit_exp(e[:, :], lps[:, :],
                                                     stg_sb)
                                        else:
                                            emit_exp(
                                                e[:, :], lps[:, :], stg_sb,
                                                bias_act=kvind_sb[
                                                    :, bias_j, 4:5],
                                                bias_pool=kvind_sb[
                                                    :, bias_j, 5:6])
                                        if mask_mj is not None:
                                            nc.vector.tensor_mul(
                                                e[:, :], e[:, :],
                                                masks_sb[:, mask_mj, :])
                                        es.append(e)
                                for g in range(4):
                                    for qi in range(4):
                                        first = g % 2 == 0 and qi == 0
                                        last = g % 2 == 1 and qi == 3
                                        mm2 = nc.tensor.matmul(
                                            oacc[g // 2][:, 4 * (g % 2) + qi,
                                                         :],
                                            es[g][:, 128 * qi:128 * qi + 128],
                                            vsrc(g),
                                            start=start and first,
                                            stop=stop and last)
                                        if record and g == 0 and qi == 0:
                                            slot_pe_readers.append(mm2.ins)
                                        nc.tensor.matmul(
                                            dacc[:, 4 * g + qi:4 * g + qi + 1],
                                            es[g][:, 128 * qi:128 * qi + 128],
                                            ones_col[:, :],
                                            start=start and g == 0 and qi == 0,
                                            stop=stop and g == 3 and qi == 3)

                            # diagonal (own-chunk) tiles, from local ks/vs
                            for tt in range(4):
                                _tile_pass(
                                    lambda lh, p: ks[64 * lh:64 * lh + 64,
                                                     2 * qr + p,
                                                     128 * tt:128 * tt + 128],
                                    lambda g, tt=tt: vs[
                                        :, tt,
                                        256 * qr + 64 * g:
                                        256 * qr + 64 * g + 64],
                                    tt, None,
                                    start=(tt == 0), stop=False)
                            # peer-slot tiles, from krecv/vrecv
                            for j in range(12):
                                d, tt = divmod(j, 4)
                                _tile_pass(
                                    lambda lh, p: krecv[
                                        64 * lh:64 * lh + 64, d,
                                        2 * qr + p,
                                        128 * tt:128 * tt + 128],
                                    lambda g, d=d, tt=tt: vrecv[
                                        :, d, tt,
                                        256 * qr + 64 * g:
                                        256 * qr + 64 * g + 64],
                                    None, j,
                                    start=False, stop=(j == 11), record=True)

                            # normalize (per-query denominator is a
                            # per-partition scalar), transpose to
                            # feature-major via PE, write mha.
                            den = n_sb.tile([128, 16], F32, tag="den")
                            nc.vector.reciprocal(den[:, :], dacc[:, :])
                            for qi in range(4):
                                for hp in range(2):
                                    # two heads side by side: one base-0
                                    # transpose lands in mha layout
                                    avp = n_sb.tile([128, 128], BF16,
                                                    tag="av")
                                    for sub in range(2):
                                        g = 2 * hp + sub
                                        nc.vector.tensor_scalar(
                                            out=avp[:, 64 * sub:
                                                    64 * sub + 64],
                                            in0=oacc[hp][:, 4 * sub + qi, :],
                                            scalar1=den[:, 4 * g + qi:
                                                        4 * g + qi + 1],
                                            scalar2=None, op0=ALU.mult)
                                    tp = t_ps.tile([128, 1024], BF16,
                                                   tag="tp")
                                    nc.tensor.transpose(
                                        tp[:, 0:128], avp[:, :], ident)
                                    if hp == 0:
                                        nc.vector.tensor_copy(
                                            out=mha[:, 2 * qr + hp,
                                                    128 * qi:128 * qi + 128],
                                            in_=tp[:, 0:128])
                                    else:
                                        nc.scalar.copy(
                                            mha[:, 2 * qr + hp,
                                                128 * qi:128 * qi + 128],
                                            tp[:, 0:128])

                            if qr == 1:
                                # w1 stripe 0 (consumed at FFN1, far away)
                                for hf_ in range(4):
                                    nc.scalar.dma_start(
                                        out=w1pre[:, 2 * hf_:2 * hf_ + 2, :],
                                        in_=w1[256 * hf_:256 * hf_ + 256,
                                               0:512]
                                        .rearrange("(a p) f -> p a f", p=128))
                            if qr == 2:
                                # prefetch WO while pass 3 computes
                                wosbs = []
                                for gh in range(2):
                                    wosb = wo_pool.tile(
                                        [128, 4, C], BF16, tag="wosb",
                                        name=f"wosb{gh}")
                                    for hf in range(2):
                                        nc.sync.dma_start(
                                            out=wosb[:, 2 * hf:2 * hf + 2, :],
                                            in_=wo[4 * gh + 2 * hf:
                                                   4 * gh + 2 * hf + 2, :, :]
                                            .rearrange("h p f -> p h f"))
                                    wosbs.append(wosb)

                if dbg is not None:
                    for sl in range(8):
                        nc.sync.dma_start(out=dbg[:, sl, :],
                                          in_=mha[:, sl, :])
                tc.tile_set_cur_wait(0, enable=True)
                # ---- WO + residual -> Z1, LN1 stats interleaved ----
                with (
                    tc.tile_pool(name="wo_ps", bufs=3, space="PSUM") as wo_ps,
                    tc.tile_pool(name="st1_ps", bufs=1,
                                 space="PSUM") as st1_ps,
                    tc.tile_pool(name="st1_sb", bufs=2) as st1_sb,
                ):
                    m1_ps = st1_ps.tile([1, 512], F32, tag="ln_m")
                    sq1_ps = st1_ps.tile([1, 512], F32, tag="ln_sq")
                    for co in range(CT):
                        wop = wo_ps.tile([128, 512], F32, tag="wop")
                        for p in range(8):
                            nc.tensor.matmul(
                                wop[:, :],
                                wosbs[p // 4][:, p % 4,
                                              128 * co:128 * co + 128],
                                mha[:, p, :],
                                start=(p == 0), stop=(p == 7))
                        nc.vector.scalar_tensor_tensor(
                            out=z1[:, co, :], in0=wop[:, :],
                            scalar=bo_sb[:, co:co + 1], in1=xq[:, co, :],
                            op0=ALU.add, op1=ALU.add)
                        _ln_stats_step(nc, st1_ps, st1_sb, m1_ps, sq1_ps,
                                       z1[:, co, :], ones_col, co,
                                       sq_eng=nc.vector)

            # ------------- LN1 -> y1 (FFN weight prefetch overlaps) ----
            z2 = post.tile([128, 8, 512], BF16, tag="z")
            with (
                tc.tile_pool(name="ffn_h", bufs=1) as ffn_h,
                tc.tile_pool(name="w1_sb", bufs=4) as w1_pool,
                tc.tile_pool(name="w2_sb", bufs=2) as w2_pool,
            ):
                def _load_w1s(s, pool=None):
                    t = (pool or w1_pool).tile(
                        [128, 8, 512], BF16, tag="w1s", name=f"w1s{s}")
                    for hf in range(4):
                        nc.sync.dma_start(
                            out=t[:, 2 * hf:2 * hf + 2, :],
                            in_=w1[256 * hf:256 * hf + 256,
                                   512 * s:512 * s + 512]
                            .rearrange("(a p) f -> p a f", p=128))
                    return t

                def _load_w2c(co):
                    t = w2_pool.tile([128, FFT, 128], BF16, tag="w2c",
                                     name=f"w2c{co}")
                    for hf in range(2):
                        nc.sync.dma_start(
                            out=t[:, 16 * hf:16 * hf + 16, :],
                            in_=w2[2048 * hf:2048 * hf + 2048,
                                   128 * co:128 * co + 128]
                            .rearrange("(a p) n -> p a n", p=128))
                    return t

                w1s_next = w1pre
                w2c_next = _load_w2c(0)
                with (
                    tc.tile_pool(name="stat_ps1", bufs=1,
                                 space="PSUM") as stat_ps,
                    tc.tile_pool(name="stat_sb1", bufs=4) as stat_sb,
                    tc.tile_pool(name="hfix_sb", bufs=6) as hfix_sb,
                ):
                    bcm1, bcr1 = _layernorm_feature_major(
                        nc, tc, persist, stat_ps, stat_sb,
                        lambda c: z1[:, c, :], y1, g1_sb, bt1_sb,
                        ones_col, ones_row, eps_t, stats=(m1_ps, sq1_ps))

                    hbuf = ffn_h.tile([128, FFT, 512], BF16)
                    # pass 1 on UNNORMALIZED z1 (gamma folded into W1 on
                    # host): h = relu(r*(h_pre - m*u) + v + b1) where
                    # u = sum_f W1g[f,:], v = sum_f W1[f,:]*beta1[f]
                    with tc.tile_pool(name="h_ps", bufs=6,
                                      space="PSUM") as h_ps:
                        for s in range(8):  # 8 stripes of 512 ff cols
                            w1s = w1s_next
                            if s < 7:
                                w1s_next = _load_w1s(s + 1)
                            for k in range(4):
                                f = 4 * s + k
                                hps = h_ps.tile([128, 512], F32, tag="hps")
                                for ci in range(CT):
                                    nc.tensor.matmul(
                                        hps[:, :],
                                        w1s[:, ci, 128 * k:128 * k + 128],
                                        z1[:, ci, :],
                                        start=(ci == 0), stop=(ci == CT - 1))
                                t1 = hfix_sb.tile([128, 512], F32,
                                                  tag="t1")
                                nc.vector.scalar_tensor_tensor(
                                    out=t1[:, :], in0=bcm1[:, :],
                                    scalar=uneg_sb[:, f:f + 1],
                                    in1=hps[:, :],
                                    op0=ALU.mult, op1=ALU.add)
                                t2 = hfix_sb.tile([128, 512], F32,
                                                  tag="t2")
                                nc.gpsimd.tensor_mul(t2[:, :], t1[:, :],
                                                     bcr1[:, :])
                                nc.scalar.activation(
                                    hbuf[:, f, :], t2[:, :], AF.Relu,
                                    bias=vb1_sb[:, f:f + 1])
                # pass 2: z2 = h @ W2 + b2 + y1, output-column major,
                # LN2 stats accumulated as each column lands
                with (
                    tc.tile_pool(name="o2_ps", bufs=3,
                                 space="PSUM") as o2_ps,
                    tc.tile_pool(name="st2_ps", bufs=1,
                                 space="PSUM") as st2_ps,
                    tc.tile_pool(name="st2_sb", bufs=2) as st2_sb,
                ):
                    m2_ps = st2_ps.tile([1, 512], F32, tag="ln_m")
                    sq2_ps = st2_ps.tile([1, 512], F32, tag="ln_sq")
                    for co in range(CT):
                        w2c = w2c_next
                        if co < CT - 1:
                            w2c_next = _load_w2c(co + 1)
                        o2t = o2_ps.tile([128, 512], F32, tag="o2")
                        for f in range(FFT):
                            nc.tensor.matmul(
                                o2t[:, :],
                                w2c[:, f, :],
                                hbuf[:, f, :],
                                start=(f == 0), stop=(f == FFT - 1))
                        nc.vector.scalar_tensor_tensor(
                            out=z2[:, co, :], in0=o2t[:, :],
                            scalar=b2_sb[:, co:co + 1], in1=y1[:, co, :],
                            op0=ALU.add, op1=ALU.add)
                        _ln_stats_step(nc, st2_ps, st2_sb, m2_ps, sq2_ps,
                                       z2[:, co, :], ones_col, co,
                                       sq_eng=nc.vector)

            # ------------- LN2 -> output -------------
            with (
                tc.tile_pool(name="stat_ps2", bufs=1, space="PSUM") as stat_ps2,
                tc.tile_pool(name="stat_sb2", bufs=6) as stat_sb2,
            ):
                y2 = post.tile([128, 8, 512], F32, tag="y")

                def _out_dma(c):
                    nc.sync.dma_start(out=out[128 * c:128 * c + 128, :],
                                      in_=y2[:, c, :])

                _layernorm_feature_major(
                    nc, tc, persist, stat_ps2, stat_sb2,
                    lambda c: z2[:, c, :], y2, g2_sb, bt2_sb,
                    ones_col, ones_row, eps_t, out_dma=_out_dma,
                    stats=(m2_ps, sq2_ps))

    # ---- post-schedule insertion of the RDMA sync protocol ----
    fn = nc.m.functions[0]

    def find_block(ins):
        for b in fn.blocks:
            if any(i is ins for i in b.instructions):
                return b
        raise KeyError(ins.name)

    def insert(ins_list, anchor, after):
        b = find_block(anchor)
        for x in ins_list:
            bb = find_block(x)
            bb.instructions.remove(x)
        idx = next(i for i, v in enumerate(b.instructions) if v is anchor)
        if after:
            idx += 1
        for x in reversed(ins_list):
            b.instructions.insert(idx, x)

    # after each trigger: drain sends (HW: blocks until DMA queues are
    # empty), then a quad barrier collective whose then_inc gates readers
    def kv_barrier(tag, trig_ins, sem):
        drn = nc.gpsimd.drain()
        b_in = nc.dram_tensor(f"kvbar_{tag}_in", [1, 1], mybir.dt.uint8)
        b_out = nc.dram_tensor(f"kvbar_{tag}_out", [4, 1], mybir.dt.uint8)
        cc = nc.gpsimd.collective_compute(
            "AllGather", ALU.bypass, RG, [b_in.ap()], [b_out.ap()])
        bass.BassInstruction(cc.ins).then_inc(sem, 1)
        insert([drn.ins, cc.ins], trig_ins, after=True)

    kv_barrier("kv", trig_v.ins, arrk_sem)

    # entry barrier: no RDMA packet may land before every quad peer has
    # entered this execution (protects recv buffers across runs); placed
    # right before the first send prep so kernel start doesn't stall on it
    ebw = nc.gpsimd.bir_kernel_barrier_wait(RG)
    cl1 = nc.gpsimd.sem_clear(arrk_sem)
    cl2 = nc.gpsimd.sem_clear(arrv_sem)
    cl3 = nc.gpsimd.sem_clear(rsem)
    cl4 = nc.gpsimd.sem_clear(lsem)
    insert([ebw.ins, cl1.ins, cl2.ins, cl3.ins, cl4.ins],
           k_preps[0].ins, after=False)

    # reader gates: PE before first krecv matmul, DVE before first
    # vrecv copy (min block index over all recorded readers)
    blk = find_block(slot_pe_readers[0])
    order = {id(v): i for i, v in enumerate(blk.instructions)}
    first_pe = min(slot_pe_readers, key=lambda i: order[id(i)])
    w_pe = nc.tensor.wait_ge(arrk_sem, 1)
    insert([w_pe.ins], first_pe, after=False)
    if slot_dve_readers:
        first_dve = min(slot_dve_readers, key=lambda i: order[id(i)])
        w_dve = nc.vector.wait_ge(arrk_sem, 1)
        insert([w_dve.ins], first_dve, after=False)

    nc.compile()
    return nc


def _prep_inputs(x, Wqkv, bqkv, WO, bO, gamma1, beta1, gamma2, beta2,
                 W1, b1, W2, b2):
    """Build the 8 per-core input maps (all host-side numpy)."""
    f32 = np.float32
    bf16 = BF16_NP
    x = np.asarray(x, f32)
    Wqkv = np.asarray(Wqkv, f32)
    bqkv = np.asarray(bqkv, f32)

    # head-major feature-ordered projection weights [C, 1024]
    wq_np = np.ascontiguousarray(
        Wqkv[:, :, 0:DK].transpose(1, 0, 2).reshape(C, C).astype(bf16))
    wk_np = np.ascontiguousarray(
        Wqkv[:, :, DK:2 * DK].transpose(1, 0, 2).reshape(C, C).astype(bf16))
    wv_np = np.ascontiguousarray(
        Wqkv[:, :, 2 * DK:3 * DK].transpose(1, 0, 2).reshape(C, C).astype(bf16))
    wo_np = np.ascontiguousarray(np.asarray(WO, f32).reshape(8, 128, C)
                                 .astype(bf16))
    w1g = np.asarray(W1, f32) * np.asarray(gamma1, f32)[:, None]
    w1_np = np.ascontiguousarray(w1g.astype(bf16))
    u_np = w1_np.astype(f32).sum(axis=0)          # sum_f W1g[f, :]
    v_np = (np.asarray(W1, f32)
            * np.asarray(beta1, f32)[:, None]).sum(axis=0)
    w2_np = np.ascontiguousarray(np.asarray(W2, f32).astype(bf16))

    def col8(v):  # [1024] -> [128, 8] (col j = elements 128j:128j+128)
        return np.ascontiguousarray(np.asarray(v, f32).reshape(8, 128).T)

    # V bias folded into the WO bias: attn weights sum to 1, so
    # mha = raw_av + bv  =>  mha@WO + bO == raw_av@WO + (bO + bv@WO).
    bv_full = bqkv[:, 2 * DK:3 * DK].reshape(C).astype(f32)
    bo_eff = np.asarray(bO, f32) + bv_full @ np.asarray(WO, f32)

    scal_np = np.zeros((128, 176), f32)
    scal_np[:, 0:8] = col8(bqkv[:, 0:DK].reshape(C))
    scal_np[:, 8:16] = col8(bqkv[:, DK:2 * DK].reshape(C))
    scal_np[:, 32:40] = col8(bo_eff)
    scal_np[:, 40:72] = np.asarray(b1, f32).reshape(32, 128).T
    scal_np[:, 72:80] = col8(b2)
    scal_np[:, 80:88] = col8(gamma1)
    scal_np[:, 88:96] = col8(beta1)
    scal_np[:, 96:104] = col8(gamma2)
    scal_np[:, 104:112] = col8(beta2)
    scal_np[:, 112:144] = (-u_np).reshape(32, 128).T
    scal_np[:, 144:176] = (v_np + np.asarray(b1, f32)).reshape(32, 128).T
    scal_np = np.ascontiguousarray(scal_np)

    # causal masks for the 4 own-chunk diagonal tiles (same on every core)
    # + identity matrix for PE transposes in slot 4
    tq = np.arange(512)[None, :]
    masks_np = np.zeros((5, 128, 512), bf16)
    for j in range(4):
        tk = (128 * j + np.arange(128))[:, None]
        masks_np[j] = (tq >= tk).astype(bf16)
    masks_np[4, :, 0:128] = np.eye(128, dtype=bf16)

    in_maps = []
    for r in range(NCORES):
        b, ch = divmod(r, 4)
        qs = QCH * ch
        xc_np = np.ascontiguousarray(x[b].T[:, qs:qs + QCH].astype(bf16))
        # peer-slot visibility: slot d-1 holds chunk (ch ^ d)
        kvind_np = np.zeros((12, 128, 8), f32)
        for d in (1, 2, 3):
            vis = 1.0 if (ch ^ d) < ch else 0.0
            kvind_np[4 * (d - 1):4 * d, :, 0:4] = vis
            # visibility as a large negative exp bias: col 4 post-scale
            # (Act Exp bias), col 5 pre-scale (added before gpsimd pow)
            kvind_np[4 * (d - 1):4 * d, :, 4] = (vis - 1.0) * 38.0
            kvind_np[4 * (d - 1):4 * d, :, 5] = (vis - 1.0) * 304.0
        in_maps.append({
            "xc": xc_np,
            "wq": wq_np, "wk": wk_np, "wv": wv_np, "wo": wo_np,
            "w1": w1_np, "w2": w2_np,
            "masks": masks_np, "kvind": np.ascontiguousarray(kvind_np),
            "scal": scal_np,
        })
    return in_maps


def kernel(**inputs):
    if "nc" not in _CACHE:
        _CACHE["nc"] = _build()
    nc = _CACHE["nc"]
    in_maps = _prep_inputs(**inputs)
    trace = os.environ.get("KERNEL_TRACE", "0") == "1"
    res = run_bass_kernel_spmd(nc, in_maps, core_ids=list(range(NCORES)),
                               trace=trace)
    _CACHE["last_result"] = res
    out = np.empty((B, T, C), np.float32)
    for r in range(NCORES):
        b, ch = divmod(r, 4)
        out[b, QCH * ch:QCH * ch + QCH, :] = res.results[r]["out"].T
    return out



# revision 41
# speedup vs baseline: 1.0078x; 1.0078x over previous
"""Trainium2 Bass kernel for a dense transformer block.

Block: x = LN1(x + MHA(x)); x = LN2(x + FFN(x))
Shapes: B=2, T=2048, C=1024, H=16, DK=64, FF=4096, fp32 in/out, bf16
internally.

Sharding: token-parallel over 8 cores (core r: batch r//4, query chunk
c = r%4 of 512 tokens, all 16 heads) with cross-core K/V sharing: each
core computes K/V for only its OWN 512 tokens and broadcasts them to
its 3 quad peers over D2D remote DMA (XOR-relative dests, so the
program is uniform SPMD). Peer slot d-1 holds chunk (c ^ d); per-core
visibility (causal masking of whole peer chunks) is data: a large
negative per-partition bias folded into the softmax exp (post-scale
for the Act engine, pre-scale during the PSUM->SBUF staging copy for
the GpSimd pow path), so invisible tiles contribute ~e^-38 to both the
AV sums and the denominators. The own chunk's diagonal tiles use
static triangle masks multiplied into the exp output. Softmax exp is
load-balanced at build time between Act (activation Exp) and GpSimd
(tensor_tensor pow with base e^(1/8); staged to SBUF by a DVE copy
since GPSIMD cannot read PSUM). AV matmuls are query-major (out
[128q, 64v], 64-row moving cost instead of 512) with denominators
accumulated by separate 1-column matmuls; each PSUM 2KB zero-region
gets exactly one start=True / stop=True (interleaved accumulation
groups in one bank corrupt on HW). Attention output is normalized by
a per-partition reciprocal multiply and PE-transposed (two heads side
by side, base partition 0) back to feature-major mha. Sync:
kernel-entry barrier, then sends -> DMA drain -> quad AllGather
barrier (then_inc) -> PE sem wait (inserted post-tile-scheduling); all
four passes' Q projections are hoisted before the gate so the 15us
collective hides under them. LayerNorm broadcasts are bf16 (4x DVE
mode) and the final affine of the f32 output LN runs on Act
(Identity with AP scale/bias).
"""

import os
import math
import numpy as np
import ml_dtypes

import concourse.bass as bass
import concourse.bass_isa as bass_isa
import concourse.mybir as mybir
import concourse.tile as tile
from concourse import bacc
from concourse.bass_utils import run_bass_kernel_spmd

BF16_NP = ml_dtypes.bfloat16

F32 = mybir.dt.float32
F32R = mybir.dt.float32r
BF16 = mybir.dt.bfloat16
AF = mybir.ActivationFunctionType
ALU = mybir.AluOpType

B, T, C = 2, 2048, 1024
H, DK = 16, 64
FF = 4 * C
EPS = 1e-5
NCORES = 8
QCH = 512            # query tokens per core (= own kv chunk)
CT = C // 128        # 8 c-tiles
FFT = FF // 128      # 32 ff-tiles
SCALE = 1.0 / math.sqrt(DK)
EBASE = math.exp(SCALE)   # exp(x*SCALE) == EBASE ** x (gpsimd pow path)
RG = [[0, 1, 2, 3], [4, 5, 6, 7]]

_CACHE = {}


def _ln_stats_step(nc, ps_pool, sb_pool, m_ps, sq_ps, z_ap, ones_col, c,
                   sq_eng=None):
    """Accumulate per-token sum and sum-of-squares for one c-tile."""
    nc.tensor.matmul(m_ps[:, :], ones_col[:, :], z_ap,
                     start=(c == 0), stop=(c == CT - 1))
    zsq = sb_pool.tile([128, 512], BF16, tag="ln_zsq")
    (sq_eng or nc.gpsimd).tensor_mul(zsq[:, :], z_ap, z_ap)
    nc.tensor.matmul(sq_ps[:, :], ones_col[:, :], zsq[:, :],
                     start=(c == 0), stop=(c == CT - 1))


def _layernorm_feature_major(nc, tc, persist, ps_pool, sb_pool, z_tiles, y_tile,
                             gamma_sb, beta_sb, ones_col, ones_row, eps_t,
                             out_dma=None, stats=None):
    """y = LN(z) over the feature axis (partitions x 8 c-tiles).

    z_tiles: callable c -> AP [128, 512] (bf16), y_tile: [128, 8, 512].
    gamma_sb/beta_sb: [128, 8] fp32. Stats per token via ones-matmuls
    (or already accumulated in `stats`=(m_ps, sq_ps)).
    """
    if stats is not None:
        m_ps, sq_ps = stats
    else:
        m_ps = ps_pool.tile([1, 512], F32, tag="ln_m")
        sq_ps = ps_pool.tile([1, 512], F32, tag="ln_sq")
        for c in range(CT):
            _ln_stats_step(nc, ps_pool, sb_pool, m_ps, sq_ps, z_tiles(c),
                           ones_col, c)
    mean_sb = sb_pool.tile([1, 512], F32R, tag="ln_mean")
    nc.vector.tensor_scalar(out=mean_sb[:, :], in0=m_ps[:, :],
                            scalar1=1.0 / C, scalar2=0.0,
                            op0=ALU.mult, op1=ALU.add)
    msq_sb = sb_pool.tile([1, 512], F32, tag="ln_msq")
    nc.vector.tensor_scalar(out=msq_sb[:, :], in0=sq_ps[:, :],
                            scalar1=1.0 / C, scalar2=0.0,
                            op0=ALU.mult, op1=ALU.add)
    var_sb = sb_pool.tile([1, 512], F32, tag="ln_var")
    nc.vector.tensor_mul(var_sb[:, :], mean_sb[:, :], mean_sb[:, :])
    nc.vector.tensor_sub(var_sb[:, :], msq_sb[:, :], var_sb[:, :])
    sd_sb = sb_pool.tile([1, 512], F32, tag="ln_sd")
    nc.scalar.activation(sd_sb[:, :], var_sb[:, :], AF.Sqrt, bias=eps_t[:, :])
    rstd_sb = sb_pool.tile([1, 512], F32R, tag="ln_rstd")
    nc.vector.reciprocal(rstd_sb[:, :], sd_sb[:, :])

    bcm_ps = ps_pool.tile([128, 512], F32, tag="ln_bcm")
    nc.tensor.matmul(bcm_ps[:, :], ones_row[0:1, :], mean_sb[:, :],
                     start=True, stop=True)
    bcr_ps = ps_pool.tile([128, 512], F32, tag="ln_bcr")
    nc.tensor.matmul(bcr_ps[:, :], ones_row[0:1, :], rstd_sb[:, :],
                     start=True, stop=True)
    # bf16 broadcasts: all-bf16 DVE ops below run in 4x mode; the ~0.4%
    # rounding on mean/rstd is well inside the error budget.
    bcm_sb = sb_pool.tile([128, 512], BF16, tag="ln_bcm_sb")
    nc.vector.tensor_copy(bcm_sb[:, :], bcm_ps[:, :])
    bcr_sb = sb_pool.tile([128, 512], BF16, tag="ln_bcr_sb")
    nc.vector.tensor_copy(bcr_sb[:, :], bcr_ps[:, :])

    act_affine = y_tile.tensor.dtype != BF16
    for c in range(CT):
        t0 = sb_pool.tile([128, 512], BF16, tag="ln_t0")
        nc.vector.tensor_sub(t0[:, :], z_tiles(c), bcm_sb[:, :])
        nc.vector.tensor_mul(t0[:, :], t0[:, :], bcr_sb[:, :])
        if act_affine:
            # f32 y (the kernel output): affine on the otherwise-idle
            # Act engine; out = Identity(t0*gamma + beta)
            nc.scalar.activation(
                y_tile[:, c, :], t0[:, :], AF.Identity,
                bias=beta_sb[:, c:c + 1], scale=gamma_sb[:, c:c + 1])
        else:
            nc.vector.tensor_scalar(
                out=y_tile[:, c, :], in0=t0[:, :],
                scalar1=gamma_sb[:, c:c + 1], scalar2=beta_sb[:, c:c + 1],
                op0=ALU.mult, op1=ALU.add)
        if out_dma is not None:
            out_dma(c)
    return bcm_sb, bcr_sb


def _build():
    nc = bacc.Bacc("TRN2", target_bir_lowering=False, debug=False,
                   num_devices=NCORES)

    xc_d = nc.dram_tensor("xc", [C, QCH], BF16, kind="ExternalInput")
    wq = nc.dram_tensor("wq", [C, C], BF16, kind="ExternalInput")
    wk = nc.dram_tensor("wk", [C, C], BF16, kind="ExternalInput")
    wv = nc.dram_tensor("wv", [C, C], BF16, kind="ExternalInput")
    wo = nc.dram_tensor("wo", [8, 128, C], BF16, kind="ExternalInput")
    w1 = nc.dram_tensor("w1", [C, FF], BF16, kind="ExternalInput")
    w2 = nc.dram_tensor("w2", [FF, C], BF16, kind="ExternalInput")
    # masks[0:4]: causal triangles; masks[4][:, 0:128]: identity (PE transp.)
    masks = nc.dram_tensor("masks", [5, 128, 512], BF16, kind="ExternalInput")
    kvind = nc.dram_tensor("kvind", [12, 128, 8], F32, kind="ExternalInput")
    scal = nc.dram_tensor("scal", [128, 176], F32, kind="ExternalInput")
    out = nc.dram_tensor("out", [C, QCH], F32, kind="ExternalOutput")
    dbg = None
    if os.environ.get("KERNEL_DEBUG", "0") == "1":
        dbg = nc.dram_tensor("dbg", [128, 8, 512], BF16,
                             kind="ExternalOutput")

    arrk_sem = nc.alloc_semaphore("k_arrived")
    arrv_sem = nc.alloc_semaphore("v_arrived")
    rsem = nc.alloc_semaphore("rdma_rsem")
    lsem = nc.alloc_semaphore("rdma_lsem")



    def bcast4(out_ap, in_ap, d):
        """remote_dma_broadcast with a 4-slot dest list (1 real XOR dest).

        Same ucode contract as the stock helper (power-of-2 n_dests, the
        RMTV ^2 lane-balance stays in range, no D2D slots needed for
        intra-device transfers) but with 4 lane-slots instead of 8.
        """
        free_b = in_ap.free_size() * mybir.dt.size(in_ap.dtype)
        packed = [-1] * 8
        packed[d] = d  # (rid=0) << 3 | tpb=d
        inst = nc.gpsimd.add_instruction(
            bass_isa.InstRemoteDMABroadcastDescs(
                name=f"I-{nc.next_id()}",
                ins=[nc.gpsimd.lower_ap(in_ap, for_isa=True)],
                outs=[nc.gpsimd.lower_ap(out_ap, for_isa=True)],
                free_dim_bytes=free_b,
                remote_sem=rsem.num,
                remote_sem_name=rsem.name,
                local_sem_update=bass.create_sync_update(lsem, 16),
                queue_num=0,
                dests=packed,
                relative=True,
            ))
        return nc.gpsimd._track_prepare_only(inst, 0)

    trig_k = trig_v = None
    k_preps = []
    slot_pe_readers = []   # PE matmuls reading krecv
    slot_dve_readers = []  # DVE ops reading vrecv

    with tile.TileContext(nc) as tc, nc.allow_low_precision(
            reason="bf16 tiles feed matmuls; fp32 accumulation in PSUM"):
        with (
            tc.tile_pool(name="persist", bufs=1) as persist,
            tc.tile_pool(name="post", bufs=1) as post,
            tc.tile_pool(name="w1pre", bufs=1) as w1pre_pool,
        ):
            # Constants / small inputs
            ones_f32 = persist.tile([128, 128], F32)
            nc.vector.memset(ones_f32[:, :], 1.0)
            ones_col = persist.tile([128, 1], BF16)
            nc.vector.tensor_copy(ones_col[:, :], ones_f32[:, 0:1])
            ones_bf = persist.tile([128, 8], BF16)
            nc.vector.tensor_copy(ones_bf[:, :], ones_f32[:, 0:8])
            ones_row = persist.tile([1, 128], F32R)
            nc.vector.tensor_copy(ones_row[:, :], ones_f32[0:1, :])
            eps_t = persist.tile([1, 1], F32)
            nc.vector.memset(eps_t[:, :], EPS)
            ebase = persist.tile([128, 512], F32)
            nc.vector.memset(ebase[:, :], EBASE)

            scal_sb = persist.tile([128, 176], F32)
            bq_sb = scal_sb[:, 0:8]
            bk_sb = scal_sb[:, 8:16]
            bv_sb = scal_sb[0:64, 16:32]
            bo_sb = scal_sb[:, 32:40]
            b1_sb = scal_sb[:, 40:72]
            b2_sb = scal_sb[:, 72:80]
            g1_sb = scal_sb[:, 80:88]
            bt1_sb = scal_sb[:, 88:96]
            g2_sb = scal_sb[:, 96:104]
            bt2_sb = scal_sb[:, 104:112]
            uneg_sb = scal_sb[:, 112:144]
            vb1_sb = scal_sb[:, 144:176]
            kvind_sb = persist.tile([128, 12, 8], F32)
            nc.gpsimd.dma_start(out=kvind_sb[:, :, :],
                                in_=kvind.rearrange("j p c -> p j c"))

            z1 = post.tile([128, 8, 512], BF16, tag="z")
            y1 = post.tile([128, 8, 512], BF16, tag="y")

            with (
                tc.tile_pool(name="span1", bufs=1) as span1,
                tc.tile_pool(name="kvbuf", bufs=1) as kvbuf,
                tc.tile_pool(name="wo_sb", bufs=2) as wo_pool,
            ):
                # Own-chunk x^T (feature-major), also the residual input.
                xq = span1.tile([128, 8, 512], BF16)
                for ci in range(CT):
                    nc.scalar.dma_start(
                        out=xq[:, ci, :],
                        in_=xc_d[128 * ci:128 * ci + 128, :])
                nc.gpsimd.dma_start(out=scal_sb[:, :], in_=scal[:, :])
                masks_sb = span1.tile([128, 5, 512], BF16)

                # K/V own + recv buffers (alive for all of attention)
                ks = kvbuf.tile([128, 8, 512], BF16)
                vs = kvbuf.tile([128, 4, 1024], BF16)
                krecv = kvbuf.tile([128, 3, 8, 512], BF16)
                vrecv = kvbuf.tile([128, 3, 4, 1024], BF16)

                # ---- K own: 8 feature slabs (head pairs) ----
                with (
                    tc.tile_pool(name="wkv", bufs=3) as wkv,
                    tc.tile_pool(name="kv_ps", bufs=2, space="PSUM") as kv_ps,
                ):
                    for s in range(8):
                        wks = wkv.tile([128, 8, 128], BF16, tag="wks")
                        nc.sync.dma_start(
                            out=wks[:, :, :],
                            in_=wk[:, 128 * s:128 * s + 128]
                            .rearrange("(a p) f -> p a f", p=128))
                        kps = kv_ps.tile([128, 512], F32, tag="kvp")
                        for ci in range(CT):
                            nc.tensor.matmul(
                                kps[:, :], wks[:, ci, :], xq[:, ci, :],
                                start=(ci == 0), stop=(ci == CT - 1))
                        nc.vector.tensor_scalar_add(
                            out=ks[:, s, :], in0=kps[:, :],
                            scalar1=bk_sb[:, s:s + 1])
                    # K broadcasts to the 3 quad peers (XOR-relative)
                    for d in (1, 2, 3):
                        k_preps.append(bcast4(krecv[:, d - 1, :, :],
                                              ks[:, :, :], d))
                    trig_k = nc.gpsimd.trigger_dma(count=None)

                    # ---- V own: 4 token tiles x 4 feature chunks ----
                    for fq in range(4):
                        wvs = wkv.tile([128, 8, 256], BF16, tag="wvs")
                        nc.sync.dma_start(
                            out=wvs[:, :, :],
                            in_=wv[:, 256 * fq:256 * fq + 256]
                            .rearrange("(a p) f -> p a f", p=128))
                        for tt in range(4):
                            vps = kv_ps.tile([128, 256], F32, tag="kvp")
                            for ci in range(CT):
                                nc.tensor.matmul(
                                    vps[:, :],
                                    xq[:, ci, 128 * tt:128 * tt + 128],
                                    wvs[:, ci, :],
                                    start=(ci == 0), stop=(ci == CT - 1))
                            nc.scalar.copy(
                                vs[:, tt, 256 * fq:256 * fq + 256],
                                vps[:, :])
                    for d in (1, 2, 3):
                        bcast4(vrecv[:, d - 1, :, :], vs[:, :, :], d)
                    trig_v = nc.gpsimd.trigger_dma(count=None)

                # MHA output, feature-major: head pair on partitions
                mha = span1.tile([128, 8, 512], BF16)

                # ------------- Attention: 4 passes of 4 heads -------------
                with (
                    tc.tile_pool(name="wq_sb", bufs=1) as wq_pool,
                    tc.tile_pool(name="attn_sb", bufs=4) as attn_sb,
                    tc.tile_pool(name="e_sb", bufs=8) as e_sb,
                    tc.tile_pool(name="stg_sb", bufs=4) as stg_sb,
                    tc.tile_pool(name="n_sb", bufs=4) as n_sb,
                ):
                    # full Wq upfront; all 4 passes' Q projections run
                    # before the peer gate so the kv-exchange barrier hides
                    # under them.
                    for mj in range(5):
                        nc.gpsimd.dma_start(
                            out=masks_sb[:, mj, :],
                            in_=masks[mj, :, :])
                    wq_sb = wq_pool.tile([128, 8, 1024], BF16, tag="wqf")
                    for qq in range(4):
                        nc.sync.dma_start(
                            out=wq_sb[:, 2 * qq:2 * qq + 2, :],
                            in_=wq[256 * qq:256 * qq + 256, :]
                            .rearrange("(a p) f -> p a f", p=128))
                    tc.tile_set_cur_wait(0.032)
                    w1pre = w1pre_pool.tile([128, 8, 512], BF16, name="w1s0")
                    qts = []
                    with tc.tile_pool(name="q_ps", bufs=2,
                                      space="PSUM") as q_ps:
                        for qr in range(4):
                            qt = attn_sb.tile([128, 2, 512], BF16, tag="qt",
                                              name=f"qt{qr}")
                            for kd in range(2):
                                qps = q_ps.tile([128, 512], F32, tag="qp")
                                for ci in range(CT):
                                    nc.tensor.matmul(
                                        qps[:, :],
                                        wq_sb[:, ci,
                                              256 * qr + 128 * kd:
                                              256 * qr + 128 * kd + 128],
                                        xq[:, ci, :],
                                        start=(ci == 0), stop=(ci == CT - 1))
                                nc.vector.tensor_scalar_add(
                                    out=qt[:, kd, :], in0=qps[:, :],
                                    scalar1=bq_sb[:, 2 * qr + kd:
                                                  2 * qr + kd + 1])
                            qts.append(qt)

                    ident = masks_sb[:, 4, 0:128]
                    # greedy build-time load balance of exp tiles between
                    # the Act engine (direct from PSUM) and GpSimd pow
                    # (staged PSUM->SBUF by DMA: GPSIMD cannot read PSUM;
                    # the DMA engines are idle during attention).
                    exp_t = [0.0, 0.0]

                    def emit_exp(e_ap, l_ap, stage_pool, bias_act=None,
                                 bias_pool=None):
                        """Visibility of whole peer kv-tiles is folded into
                        the exp as a large negative per-partition bias
                        (bias_act post-scale for Act, bias_pool pre-scale
                        added during the PSUM->SBUF staging copy)."""
                        if exp_t[0] + 612 <= exp_t[1] + 1167:
                            exp_t[0] += 612
                            nc.scalar.activation(
                                e_ap, l_ap, AF.Exp, scale=SCALE,
                                bias=bias_act if bias_act is not None
                                else 0.0)
                        else:
                            exp_t[1] += 1167
                            stg = stage_pool.tile([128, 512], F32,
                                                  tag="pstg")
                            if bias_pool is None:
                                nc.vector.tensor_copy(stg[:, :], l_ap)
                            else:
                                nc.vector.tensor_scalar(
                                    out=stg[:, :], in0=l_ap,
                                    scalar1=bias_pool, scalar2=None,
                                    op0=ALU.add)
                            nc.gpsimd.tensor_tensor(e_ap, ebase[:, :],
                                                    stg[:, :], ALU.pow)

                    with (
                        tc.tile_pool(name="l_ps", bufs=4,
                                     space="PSUM") as l_ps,
                        tc.tile_pool(name="o_ps", bufs=1,
                                     space="PSUM") as o_ps,
                        tc.tile_pool(name="t_ps", bufs=1,
                                     space="PSUM") as t_ps,
                    ):
                        def emit_normalize(qr_p, oacc_p, dacc_p):
                            # per-query denominator is a per-partition
                            # scalar; PE-transpose two heads side by side
                            # back to feature-major mha
                            den = n_sb.tile([128, 16], F32, tag="den")
                            nc.vector.reciprocal(den[:, :], dacc_p[:, :])
                            for qi in range(4):
                                for hp in range(2):
                                    avp = n_sb.tile([128, 128], BF16,
                                                    tag="av")
                                    for sub in range(2):
                                        g = 2 * hp + sub
                                        nc.vector.tensor_scalar(
                                            out=avp[:, 64 * sub:
                                                    64 * sub + 64],
                                            in0=oacc_p[hp][:, 4 * sub + qi,
                                                           :],
                                            scalar1=den[:, 4 * g + qi:
                                                        4 * g + qi + 1],
                                            scalar2=None, op0=ALU.mult)
                                    tp = t_ps.tile([128, 1024], BF16,
                                                   tag="tp")
                                    nc.tensor.transpose(
                                        tp[:, 0:128], avp[:, :], ident)
                                    if hp == 0:
                                        nc.vector.tensor_copy(
                                            out=mha[:, 2 * qr_p + hp,
                                                    128 * qi:128 * qi + 128],
                                            in_=tp[:, 0:128])
                                    else:
                                        nc.scalar.copy(
                                            mha[:, 2 * qr_p + hp,
                                                128 * qi:128 * qi + 128],
                                            tp[:, 0:128])

                        pending = None
                        for qr in range(4):
                            qt = qts[qr]
                            # AV accumulators: head pair x 4 query tiles x
                            # 64 feats (query-major), one PSUM bank each;
                            # softmax denominators accumulate separately in
                            # a shared [128, 16] bank.
                            # One PSUM zero-region (2KB bank) per tile:
                            # exactly ONE start=True per region (first
                            # matmul), first-touch writes of other slots
                            # overwrite via pending-zero, one stop=True on
                            # the region's last matmul.
                            oacc = [o_ps.tile([128, 8, 64], F32,
                                              tag=f"o{hp}",
                                              name=f"o{hp}_{qr}")
                                    for hp in range(2)]
                            dacc_pad = o_ps.tile([128, 512], F32,
                                                 tag="dacc",
                                                 name=f"dacc_{qr}")
                            dacc = dacc_pad[:, 0:16]

                            def _av_part(es, vsrc, start, stop, record):
                                for g in range(4):
                                    for qi in range(4):
                                        first = g % 2 == 0 and qi == 0
                                        last = g % 2 == 1 and qi == 3
                                        mm2 = nc.tensor.matmul(
                                            oacc[g // 2][:, 4 * (g % 2) + qi,
                                                         :],
                                            es[g][:, 128 * qi:128 * qi + 128],
                                            vsrc(g),
                                            start=start and first,
                                            stop=stop and last)
                                        if record and g == 0 and qi == 0:
                                            slot_pe_readers.append(mm2.ins)
                                        nc.tensor.matmul(
                                            dacc[:, 4 * g + qi:4 * g + qi + 1],
                                            es[g][:, 128 * qi:128 * qi + 128],
                                            ones_col[:, :],
                                            start=start and g == 0 and qi == 0,
                                            stop=stop and g == 3 and qi == 3)

                            def _tile_pass(ksrc, vsrc, mask_mj, bias_j,
                                           start, stop, record=False,
                                           emit_av=True):
                                """One 128-token kv tile: logits+exp+AV.

                                ksrc: (lh, p) -> lhsT AP [64, 128]
                                vsrc: g -> AP [128, 64] (head g V tile)
                                mask_mj: diag mask index or None
                                bias_j: peer-slot index for visibility
                                bias columns, or None (diag, visible)
                                emit_av=False: return a thunk emitting
                                the AV matmuls later (pass-boundary
                                overlap: logits+exp of the next pass run
                                while the previous pass normalizes).
                                """
                                es = []
                                for p in range(2):
                                    for lh in range(2):
                                        lps = l_ps.tile([128, 512], F32,
                                                        tag="lg")
                                        mm = nc.tensor.matmul(
                                            lps[:, :], ksrc(lh, p),
                                            qt[64 * lh:64 * lh + 64, p, :],
                                            start=True, stop=True,
                                            tile_position=(64 * lh, 0))
                                        if record:
                                            slot_pe_readers.append(mm.ins)
                                        e = e_sb.tile([128, 512], BF16,
                                                      tag="e",
                                                      name=f"e{p}{lh}")
                                        if bias_j is None:
                                            emit_exp(e[:, :], lps[:, :],
                                                     stg_sb)
                                        else:
                                            emit_exp(
                                                e[:, :], lps[:, :], stg_sb,
                                                bias_act=kvind_sb[
                                                    :, bias_j, 4:5],
                                                bias_pool=kvind_sb[
                                                    :, bias_j, 5:6])
                                        if mask_mj is not None:
                                            nc.vector.tensor_mul(
                                                e[:, :], e[:, :],
                                                masks_sb[:, mask_mj, :])
                                        es.append(e)
                                if emit_av:
                                    _av_part(es, vsrc, start, stop, record)
                                    return None
                                return lambda: _av_part(es, vsrc, start,
                                                        stop, record)

                            def _diag_args(tt):
                                return (
                                    lambda lh, p: ks[64 * lh:64 * lh + 64,
                                                     2 * qr + p,
                                                     128 * tt:128 * tt + 128],
                                    lambda g: vs[
                                        :, tt,
                                        256 * qr + 64 * g:
                                        256 * qr + 64 * g + 64],
                                    tt, None)

                            # first two diagonal tiles: logits+exp only,
                            # so Act/Pool stay fed while the previous
                            # pass's normalize runs on DVE/PE
                            av0 = _tile_pass(*_diag_args(0), start=True,
                                             stop=False, emit_av=False)
                            av1 = _tile_pass(*_diag_args(1), start=False,
                                             stop=False, emit_av=False)
                            if pending is not None:
                                emit_normalize(*pending)
                            av0()
                            av1()
                            for tt in range(2, 4):
                                _tile_pass(*_diag_args(tt), start=False,
                                           stop=False)
                            # peer-slot tiles, from krecv/vrecv
                            for j in range(12):
                                d, tt = divmod(j, 4)
                                _tile_pass(
                                    lambda lh, p: krecv[
                                        64 * lh:64 * lh + 64, d,
                                        2 * qr + p,
                                        128 * tt:128 * tt + 128],
                                    lambda g, d=d, tt=tt: vrecv[
                                        :, d, tt,
                                        256 * qr + 64 * g:
                                        256 * qr + 64 * g + 64],
                                    None, j,
                                    start=False, stop=(j == 11), record=True)
                            pending = (qr, oacc, dacc)

                            if qr == 1:
                                # w1 stripe 0 (consumed at FFN1, far away)
                                for hf_ in range(4):
                                    nc.scalar.dma_start(
                                        out=w1pre[:, 2 * hf_:2 * hf_ + 2, :],
                                        in_=w1[256 * hf_:256 * hf_ + 256,
                                               0:512]
                                        .rearrange("(a p) f -> p a f", p=128))
                            if qr == 2:
                                # prefetch WO while pass 3 computes
                                wosbs = []
                                for gh in range(2):
                                    wosb = wo_pool.tile(
                                        [128, 4, C], BF16, tag="wosb",
                                        name=f"wosb{gh}")
                                    for hf in range(2):
                                        nc.sync.dma_start(
                                            out=wosb[:, 2 * hf:2 * hf + 2, :],
                                            in_=wo[4 * gh + 2 * hf:
                                                   4 * gh + 2 * hf + 2, :, :]
                                            .rearrange("h p f -> p h f"))
                                    wosbs.append(wosb)

                        emit_normalize(*pending)

                if dbg is not None:
                    for sl in range(8):
                        nc.sync.dma_start(out=dbg[:, sl, :],
                                          in_=mha[:, sl, :])
                tc.tile_set_cur_wait(0, enable=True)
                # ---- WO + residual -> Z1, LN1 stats interleaved ----
                with (
                    tc.tile_pool(name="wo_ps", bufs=3, space="PSUM") as wo_ps,
                    tc.tile_pool(name="st1_ps", bufs=1,
                                 space="PSUM") as st1_ps,
                    tc.tile_pool(name="st1_sb", bufs=2) as st1_sb,
                ):
                    m1_ps = st1_ps.tile([1, 512], F32, tag="ln_m")
                    sq1_ps = st1_ps.tile([1, 512], F32, tag="ln_sq")
                    for co in range(CT):
                        wop = wo_ps.tile([128, 512], F32, tag="wop")
                        for p in range(8):
                            nc.tensor.matmul(
                                wop[:, :],
                                wosbs[p // 4][:, p % 4,
                                              128 * co:128 * co + 128],
                                mha[:, p, :],
                                start=(p == 0), stop=(p == 7))
                        nc.vector.scalar_tensor_tensor(
                            out=z1[:, co, :], in0=wop[:, :],
                            scalar=bo_sb[:, co:co + 1], in1=xq[:, co, :],
                            op0=ALU.add, op1=ALU.add)
                        _ln_stats_step(nc, st1_ps, st1_sb, m1_ps, sq1_ps,
                                       z1[:, co, :], ones_col, co,
                                       sq_eng=nc.vector)

            # ------------- LN1 -> y1 (FFN weight prefetch overlaps) ----
            z2 = post.tile([128, 8, 512], BF16, tag="z")
            with (
                tc.tile_pool(name="ffn_h", bufs=1) as ffn_h,
                tc.tile_pool(name="w1_sb", bufs=4) as w1_pool,
                tc.tile_pool(name="w2_sb", bufs=2) as w2_pool,
            ):
                def _load_w1s(s, pool=None):
                    t = (pool or w1_pool).tile(
                        [128, 8, 512], BF16, tag="w1s", name=f"w1s{s}")
                    for hf in range(4):
                        nc.sync.dma_start(
                            out=t[:, 2 * hf:2 * hf + 2, :],
                            in_=w1[256 * hf:256 * hf + 256,
                                   512 * s:512 * s + 512]
                            .rearrange("(a p) f -> p a f", p=128))
                    return t

                def _load_w2c(co):
                    t = w2_pool.tile([128, FFT, 128], BF16, tag="w2c",
                                     name=f"w2c{co}")
                    for hf in range(2):
                        nc.sync.dma_start(
                            out=t[:, 16 * hf:16 * hf + 16, :],
                            in_=w2[2048 * hf:2048 * hf + 2048,
                                   128 * co:128 * co + 128]
                            .rearrange("(a p) n -> p a n", p=128))
                    return t

                w1s_next = w1pre
                w2c_next = _load_w2c(0)
                with (
                    tc.tile_pool(name="stat_ps1", bufs=1,
                                 space="PSUM") as stat_ps,
                    tc.tile_pool(name="stat_sb1", bufs=4) as stat_sb,
                    tc.tile_pool(name="hfix_sb", bufs=6) as hfix_sb,
                ):
                    bcm1, bcr1 = _layernorm_feature_major(
                        nc, tc, persist, stat_ps, stat_sb,
                        lambda c: z1[:, c, :], y1, g1_sb, bt1_sb,
                        ones_col, ones_row, eps_t, stats=(m1_ps, sq1_ps))

                    hbuf = ffn_h.tile([128, FFT, 512], BF16)
                    # pass 1 on UNNORMALIZED z1 (gamma folded into W1 on
                    # host): h = relu(r*(h_pre - m*u) + v + b1) where
                    # u = sum_f W1g[f,:], v = sum_f W1[f,:]*beta1[f]
                    with tc.tile_pool(name="h_ps", bufs=6,
                                      space="PSUM") as h_ps:
                        for s in range(8):  # 8 stripes of 512 ff cols
                            w1s = w1s_next
                            if s < 7:
                                w1s_next = _load_w1s(s + 1)
                            for k in range(4):
                                f = 4 * s + k
                                hps = h_ps.tile([128, 512], F32, tag="hps")
                                for ci in range(CT):
                                    nc.tensor.matmul(
                                        hps[:, :],
                                        w1s[:, ci, 128 * k:128 * k + 128],
                                        z1[:, ci, :],
                                        start=(ci == 0), stop=(ci == CT - 1))
                                t1 = hfix_sb.tile([128, 512], F32,
                                                  tag="t1")
                                nc.vector.scalar_tensor_tensor(
                                    out=t1[:, :], in0=bcm1[:, :],
                                    scalar=uneg_sb[:, f:f + 1],
                                    in1=hps[:, :],
                                    op0=ALU.mult, op1=ALU.add)
                                t2 = hfix_sb.tile([128, 512], F32,
                                                  tag="t2")
                                nc.gpsimd.tensor_mul(t2[:, :], t1[:, :],
                                                     bcr1[:, :])
                                nc.scalar.activation(
                                    hbuf[:, f, :], t2[:, :], AF.Relu,
                                    bias=vb1_sb[:, f:f + 1])
                # pass 2: z2 = h @ W2 + b2 + y1, output-column major,
                # LN2 stats accumulated as each column lands
                with (
                    tc.tile_pool(name="o2_ps", bufs=3,
                                 space="PSUM") as o2_ps,
                    tc.tile_pool(name="st2_ps", bufs=1,
                                 space="PSUM") as st2_ps,
                    tc.tile_pool(name="st2_sb", bufs=2) as st2_sb,
                ):
                    m2_ps = st2_ps.tile([1, 512], F32, tag="ln_m")
                    sq2_ps = st2_ps.tile([1, 512], F32, tag="ln_sq")
                    for co in range(CT):
                        w2c = w2c_next
                        if co < CT - 1:
                            w2c_next = _load_w2c(co + 1)
                        o2t = o2_ps.tile([128, 512], F32, tag="o2")
                        for f in range(FFT):
                            nc.tensor.matmul(
                                o2t[:, :],
                                w2c[:, f, :],
                                hbuf[:, f, :],
                                start=(f == 0), stop=(f == FFT - 1))
                        nc.vector.scalar_tensor_tensor(
                            out=z2[:, co, :], in0=o2t[:, :],
                            scalar=b2_sb[:, co:co + 1], in1=y1[:, co, :],
                            op0=ALU.add, op1=ALU.add)
                        _ln_stats_step(nc, st2_ps, st2_sb, m2_ps, sq2_ps,
                                       z2[:, co, :], ones_col, co,
                                       sq_eng=nc.vector)

            # ------------- LN2 -> output -------------
            with (
                tc.tile_pool(name="stat_ps2", bufs=1, space="PSUM") as stat_ps2,
                tc.tile_pool(name="stat_sb2", bufs=6) as stat_sb2,
            ):
                y2 = post.tile([128, 8, 512], F32, tag="y")

                def _out_dma(c):
                    nc.sync.dma_start(out=out[128 * c:128 * c + 128, :],
                                      in_=y2[:, c, :])

                _layernorm_feature_major(
                    nc, tc, persist, stat_ps2, stat_sb2,
                    lambda c: z2[:, c, :], y2, g2_sb, bt2_sb,
                    ones_col, ones_row, eps_t, out_dma=_out_dma,
                    stats=(m2_ps, sq2_ps))

    # ---- post-schedule insertion of the RDMA sync protocol ----
    fn = nc.m.functions[0]

    def find_block(ins):
        for b in fn.blocks:
            if any(i is ins for i in b.instructions):
                return b
        raise KeyError(ins.name)

    def insert(ins_list, anchor, after):
        b = find_block(anchor)
        for x in ins_list:
            bb = find_block(x)
            bb.instructions.remove(x)
        idx = next(i for i, v in enumerate(b.instructions) if v is anchor)
        if after:
            idx += 1
        for x in reversed(ins_list):
            b.instructions.insert(idx, x)

    # after each trigger: drain sends (HW: blocks until DMA queues are
    # empty), then a quad barrier collective whose then_inc gates readers
    def kv_barrier(tag, trig_ins, sem):
        drn = nc.gpsimd.drain()
        b_in = nc.dram_tensor(f"kvbar_{tag}_in", [1, 1], mybir.dt.uint8)
        b_out = nc.dram_tensor(f"kvbar_{tag}_out", [4, 1], mybir.dt.uint8)
        cc = nc.gpsimd.collective_compute(
            "AllGather", ALU.bypass, RG, [b_in.ap()], [b_out.ap()])
        bass.BassInstruction(cc.ins).then_inc(sem, 1)
        insert([drn.ins, cc.ins], trig_ins, after=True)

    kv_barrier("kv", trig_v.ins, arrk_sem)

    # entry barrier: no RDMA packet may land before every quad peer has
    # entered this execution (protects recv buffers across runs); placed
    # right before the first send prep so kernel start doesn't stall on it
    ebw = nc.gpsimd.bir_kernel_barrier_wait(RG)
    cl1 = nc.gpsimd.sem_clear(arrk_sem)
    cl2 = nc.gpsimd.sem_clear(arrv_sem)
    cl3 = nc.gpsimd.sem_clear(rsem)
    cl4 = nc.gpsimd.sem_clear(lsem)
    insert([ebw.ins, cl1.ins, cl2.ins, cl3.ins, cl4.ins],
           k_preps[0].ins, after=False)

    # reader gates: PE before first krecv matmul, DVE before first
    # vrecv copy (min block index over all recorded readers)
    blk = find_block(slot_pe_readers[0])
    order = {id(v): i for i, v in enumerate(blk.instructions)}
    first_pe = min(slot_pe_readers, key=lambda i: order[id(i)])
    w_pe = nc.tensor.wait_ge(arrk_sem, 1)
    insert([w_pe.ins], first_pe, after=False)
    if slot_dve_readers:
        first_dve = min(slot_dve_readers, key=lambda i: order[id(i)])
        w_dve = nc.vector.wait_ge(arrk_sem, 1)
        insert([w_dve.ins], first_dve, after=False)

    nc.compile()
    return nc


def _prep_inputs(x, Wqkv, bqkv, WO, bO, gamma1, beta1, gamma2, beta2,
                 W1, b1, W2, b2):
    """Build the 8 per-core input maps (all host-side numpy)."""
    f32 = np.float32
    bf16 = BF16_NP
    x = np.asarray(x, f32)
    Wqkv = np.asarray(Wqkv, f32)
    bqkv = np.asarray(bqkv, f32)

    # head-major feature-ordered projection weights [C, 1024]
    wq_np = np.ascontiguousarray(
        Wqkv[:, :, 0:DK].transpose(1, 0, 2).reshape(C, C).astype(bf16))
    wk_np = np.ascontiguousarray(
        Wqkv[:, :, DK:2 * DK].transpose(1, 0, 2).reshape(C, C).astype(bf16))
    wv_np = np.ascontiguousarray(
        Wqkv[:, :, 2 * DK:3 * DK].transpose(1, 0, 2).reshape(C, C).astype(bf16))
    wo_np = np.ascontiguousarray(np.asarray(WO, f32).reshape(8, 128, C)
                                 .astype(bf16))
    w1g = np.asarray(W1, f32) * np.asarray(gamma1, f32)[:, None]
    w1_np = np.ascontiguousarray(w1g.astype(bf16))
    u_np = w1_np.astype(f32).sum(axis=0)          # sum_f W1g[f, :]
    v_np = (np.asarray(W1, f32)
            * np.asarray(beta1, f32)[:, None]).sum(axis=0)
    w2_np = np.ascontiguousarray(np.asarray(W2, f32).astype(bf16))

    def col8(v):  # [1024] -> [128, 8] (col j = elements 128j:128j+128)
        return np.ascontiguousarray(np.asarray(v, f32).reshape(8, 128).T)

    # V bias folded into the WO bias: attn weights sum to 1, so
    # mha = raw_av + bv  =>  mha@WO + bO == raw_av@WO + (bO + bv@WO).
    bv_full = bqkv[:, 2 * DK:3 * DK].reshape(C).astype(f32)
    bo_eff = np.asarray(bO, f32) + bv_full @ np.asarray(WO, f32)

    scal_np = np.zeros((128, 176), f32)
    scal_np[:, 0:8] = col8(bqkv[:, 0:DK].reshape(C))
    scal_np[:, 8:16] = col8(bqkv[:, DK:2 * DK].reshape(C))
    scal_np[:, 32:40] = col8(bo_eff)
    scal_np[:, 40:72] = np.asarray(b1, f32).reshape(32, 128).T
    scal_np[:, 72:80] = col8(b2)
    scal_np[:, 80:88] = col8(gamma1)
    scal_np[:, 88:96] = col8(beta1)
    scal_np[:, 96:104] = col8(gamma2)
    scal_np[:, 104:112] = col8(beta2)
    scal_np[:, 112:144] = (-u_np).reshape(32, 128).T
    scal_np[:, 144:176] = (v_np + np.asarray(b1, f32)).reshape(32, 128).T
    scal_np = np.ascontiguousarray(scal_np)

    # causal masks for the 4 own-chunk diagonal tiles (same on every core)
    # + identity matrix for PE transposes in slot 4
    tq = np.arange(512)[None, :]
    masks_np = np.zeros((5, 128, 512), bf16)
    for j in range(4):
        tk = (128 * j + np.arange(128))[:, None]
        masks_np[j] = (tq >= tk).astype(bf16)
    masks_np[4, :, 0:128] = np.eye(128, dtype=bf16)

    in_maps = []
    for r in range(NCORES):
        b, ch = divmod(r, 4)
        qs = QCH * ch
        xc_np = np.ascontiguousarray(x[b].T[:, qs:qs + QCH].astype(bf16))
        # peer-slot visibility: slot d-1 holds chunk (ch ^ d)
        kvind_np = np.zeros((12, 128, 8), f32)
        for d in (1, 2, 3):
            vis = 1.0 if (ch ^ d) < ch else 0.0
            kvind_np[4 * (d - 1):4 * d, :, 0:4] = vis
            # visibility as a large negative exp bias: col 4 post-scale
            # (Act Exp bias), col 5 pre-scale (added before gpsimd pow)
            kvind_np[4 * (d - 1):4 * d, :, 4] = (vis - 1.0) * 38.0
            kvind_np[4 * (d - 1):4 * d, :, 5] = (vis - 1.0) * 304.0
        in_maps.append({
            "xc": xc_np,
            "wq": wq_np, "wk": wk_np, "wv": wv_np, "wo": wo_np,
            "w1": w1_np, "w2": w2_np,
            "masks": masks_np, "kvind": np.ascontiguousarray(kvind_np),
            "scal": scal_np,
        })
    return in_maps


def kernel(**inputs):
    if "nc" not in _CACHE:
        _CACHE["nc"] = _build()
    nc = _CACHE["nc"]
    in_maps = _prep_inputs(**inputs)
    trace = os.environ.get("KERNEL_TRACE", "0") == "1"
    res = run_bass_kernel_spmd(nc, in_maps, core_ids=list(range(NCORES)),
                               trace=trace)
    _CACHE["last_result"] = res
    out = np.empty((B, T, C), np.float32)
    for r in range(NCORES):
        b, ch = divmod(r, 4)
        out[b, QCH * ch:QCH * ch + QCH, :] = res.results[r]["out"].T
    return out



# revision 46
# speedup vs baseline: 1.0136x; 1.0058x over previous
"""Trainium2 Bass kernel for a dense transformer block.

Block: x = LN1(x + MHA(x)); x = LN2(x + FFN(x))
Shapes: B=2, T=2048, C=1024, H=16, DK=64, FF=4096, fp32 in/out, bf16
internally.

Sharding: token-parallel over 8 cores (core r: batch r//4, query chunk
c = r%4 of 512 tokens, all 16 heads) with cross-core K/V sharing: each
core computes K/V for only its OWN 512 tokens and broadcasts them to
its 3 quad peers over D2D remote DMA (XOR-relative dests, so the
program is uniform SPMD). Peer slot d-1 holds chunk (c ^ d); per-core
visibility (causal masking of whole peer chunks) is data: a large
negative per-partition bias folded into the softmax exp (post-scale
for the Act engine, pre-scale during the PSUM->SBUF staging copy for
the GpSimd pow path), so invisible tiles contribute ~e^-38 to both the
AV sums and the denominators. The own chunk's diagonal tiles use
static triangle masks multiplied into the exp output. Softmax exp is
load-balanced at build time between Act (activation Exp) and GpSimd
(tensor_tensor pow with base e^(1/8); staged to SBUF by a DVE copy
since GPSIMD cannot read PSUM). AV matmuls are query-major (out
[128q, 64v], 64-row moving cost instead of 512) with denominators
accumulated by separate 1-column matmuls; each PSUM 2KB zero-region
gets exactly one start=True / stop=True (interleaved accumulation
groups in one bank corrupt on HW). Attention output is normalized by
a per-partition reciprocal multiply and PE-transposed (two heads side
by side, base partition 0) back to feature-major mha. Sync:
kernel-entry barrier, then sends -> DMA drain -> quad AllGather
barrier (then_inc) -> PE sem wait (inserted post-tile-scheduling); all
four passes' Q projections are hoisted before the gate so the 15us
collective hides under them. LayerNorm broadcasts are bf16 (4x DVE
mode) and the final affine of the f32 output LN runs on Act
(Identity with AP scale/bias).
"""

import os
import math
import numpy as np
import ml_dtypes

import concourse.bass as bass
import concourse.bass_isa as bass_isa
import concourse.mybir as mybir
import concourse.tile as tile
from concourse import bacc
from concourse.bass_utils import run_bass_kernel_spmd

BF16_NP = ml_dtypes.bfloat16

F32 = mybir.dt.float32
F32R = mybir.dt.float32r
BF16 = mybir.dt.bfloat16
AF = mybir.ActivationFunctionType
ALU = mybir.AluOpType

B, T, C = 2, 2048, 1024
H, DK = 16, 64
FF = 4 * C
EPS = 1e-5
NCORES = 8
QCH = 512            # query tokens per core (= own kv chunk)
CT = C // 128        # 8 c-tiles
FFT = FF // 128      # 32 ff-tiles
SCALE = 1.0 / math.sqrt(DK)
EBASE = math.exp(SCALE)   # exp(x*SCALE) == EBASE ** x (gpsimd pow path)
RG = [[0, 1, 2, 3], [4, 5, 6, 7]]

_CACHE = {}


def _ln_stats_step(nc, ps_pool, sb_pool, m_ps, sq_ps, z_ap, ones_col, c,
                   sq_eng=None):
    """Accumulate per-token sum and sum-of-squares for one c-tile."""
    nc.tensor.matmul(m_ps[:, :], ones_col[:, :], z_ap,
                     start=(c == 0), stop=(c == CT - 1))
    zsq = sb_pool.tile([128, 512], BF16, tag="ln_zsq")
    (sq_eng or nc.gpsimd).tensor_mul(zsq[:, :], z_ap, z_ap)
    nc.tensor.matmul(sq_ps[:, :], ones_col[:, :], zsq[:, :],
                     start=(c == 0), stop=(c == CT - 1))


def _layernorm_feature_major(nc, tc, persist, ps_pool, sb_pool, z_tiles, y_tile,
                             gamma_sb, beta_sb, ones_col, ones_row, eps_t,
                             out_dma=None, stats=None):
    """y = LN(z) over the feature axis (partitions x 8 c-tiles).

    z_tiles: callable c -> AP [128, 512] (bf16), y_tile: [128, 8, 512].
    gamma_sb/beta_sb: [128, 8] fp32. Stats per token via ones-matmuls
    (or already accumulated in `stats`=(m_ps, sq_ps)).
    """
    if stats is not None:
        m_ps, sq_ps = stats
    else:
        m_ps = ps_pool.tile([1, 512], F32, tag="ln_m")
        sq_ps = ps_pool.tile([1, 512], F32, tag="ln_sq")
        for c in range(CT):
            _ln_stats_step(nc, ps_pool, sb_pool, m_ps, sq_ps, z_tiles(c),
                           ones_col, c)
    mean_sb = sb_pool.tile([1, 512], F32R, tag="ln_mean")
    nc.vector.tensor_scalar(out=mean_sb[:, :], in0=m_ps[:, :],
                            scalar1=1.0 / C, scalar2=0.0,
                            op0=ALU.mult, op1=ALU.add)
    msq_sb = sb_pool.tile([1, 512], F32, tag="ln_msq")
    nc.vector.tensor_scalar(out=msq_sb[:, :], in0=sq_ps[:, :],
                            scalar1=1.0 / C, scalar2=0.0,
                            op0=ALU.mult, op1=ALU.add)
    var_sb = sb_pool.tile([1, 512], F32, tag="ln_var")
    nc.vector.tensor_mul(var_sb[:, :], mean_sb[:, :], mean_sb[:, :])
    nc.vector.tensor_sub(var_sb[:, :], msq_sb[:, :], var_sb[:, :])
    sd_sb = sb_pool.tile([1, 512], F32, tag="ln_sd")
    nc.scalar.activation(sd_sb[:, :], var_sb[:, :], AF.Sqrt, bias=eps_t[:, :])
    rstd_sb = sb_pool.tile([1, 512], F32R, tag="ln_rstd")
    nc.vector.reciprocal(rstd_sb[:, :], sd_sb[:, :])

    bcm_ps = ps_pool.tile([128, 512], F32, tag="ln_bcm")
    nc.tensor.matmul(bcm_ps[:, :], ones_row[0:1, :], mean_sb[:, :],
                     start=True, stop=True)
    bcr_ps = ps_pool.tile([128, 512], F32, tag="ln_bcr")
    nc.tensor.matmul(bcr_ps[:, :], ones_row[0:1, :], rstd_sb[:, :],
                     start=True, stop=True)
    # bf16 broadcasts: all-bf16 DVE ops below run in 4x mode; the ~0.4%
    # rounding on mean/rstd is well inside the error budget.
    bcm_sb = sb_pool.tile([128, 512], BF16, tag="ln_bcm_sb")
    nc.vector.tensor_copy(bcm_sb[:, :], bcm_ps[:, :])
    bcr_sb = sb_pool.tile([128, 512], BF16, tag="ln_bcr_sb")
    nc.vector.tensor_copy(bcr_sb[:, :], bcr_ps[:, :])

    act_affine = y_tile.tensor.dtype != BF16
    for c in range(CT):
        t0 = sb_pool.tile([128, 512], BF16, tag="ln_t0")
        nc.vector.tensor_sub(t0[:, :], z_tiles(c), bcm_sb[:, :])
        nc.vector.tensor_mul(t0[:, :], t0[:, :], bcr_sb[:, :])
        if act_affine:
            # f32 y (the kernel output): affine on the otherwise-idle
            # Act engine; out = Identity(t0*gamma + beta)
            nc.scalar.activation(
                y_tile[:, c, :], t0[:, :], AF.Identity,
                bias=beta_sb[:, c:c + 1], scale=gamma_sb[:, c:c + 1])
        else:
            nc.vector.tensor_scalar(
                out=y_tile[:, c, :], in0=t0[:, :],
                scalar1=gamma_sb[:, c:c + 1], scalar2=beta_sb[:, c:c + 1],
                op0=ALU.mult, op1=ALU.add)
        if out_dma is not None:
            out_dma(c)
    return bcm_sb, bcr_sb


def _build():
    nc = bacc.Bacc("TRN2", target_bir_lowering=False, debug=False,
                   num_devices=NCORES)

    xc_d = nc.dram_tensor("xc", [C, QCH], BF16, kind="ExternalInput")
    wq = nc.dram_tensor("wq", [C, C], BF16, kind="ExternalInput")
    wk = nc.dram_tensor("wk", [4, C, 256], BF16, kind="ExternalInput")
    wv = nc.dram_tensor("wv", [C, C], BF16, kind="ExternalInput")
    wo = nc.dram_tensor("wo", [8, 128, C], BF16, kind="ExternalInput")
    w1 = nc.dram_tensor("w1", [C, FF], BF16, kind="ExternalInput")
    w2 = nc.dram_tensor("w2", [FF, C], BF16, kind="ExternalInput")
    # masks[0:4]: causal triangles; masks[4][:, 0:128]: identity (PE transp.)
    masks = nc.dram_tensor("masks", [5, 128, 512], BF16, kind="ExternalInput")
    kvind = nc.dram_tensor("kvind", [12, 128, 8], F32, kind="ExternalInput")
    scal = nc.dram_tensor("scal", [128, 176], F32, kind="ExternalInput")
    out = nc.dram_tensor("out", [C, QCH], F32, kind="ExternalOutput")
    dbg = None
    if os.environ.get("KERNEL_DEBUG", "0") == "1":
        dbg = nc.dram_tensor("dbg", [128, 8, 512], BF16,
                             kind="ExternalOutput")

    arrk_sem = nc.alloc_semaphore("k_arrived")
    arrv_sem = nc.alloc_semaphore("v_arrived")
    rsem = nc.alloc_semaphore("rdma_rsem")
    lsem = nc.alloc_semaphore("rdma_lsem")



    def bcast4(out_ap, in_ap, d):
        """remote_dma_broadcast with a 4-slot dest list (1 real XOR dest).

        Same ucode contract as the stock helper (power-of-2 n_dests, the
        RMTV ^2 lane-balance stays in range, no D2D slots needed for
        intra-device transfers) but with 4 lane-slots instead of 8.
        """
        free_b = in_ap.free_size() * mybir.dt.size(in_ap.dtype)
        packed = [-1] * 8
        packed[d] = d  # (rid=0) << 3 | tpb=d
        inst = nc.gpsimd.add_instruction(
            bass_isa.InstRemoteDMABroadcastDescs(
                name=f"I-{nc.next_id()}",
                ins=[nc.gpsimd.lower_ap(in_ap, for_isa=True)],
                outs=[nc.gpsimd.lower_ap(out_ap, for_isa=True)],
                free_dim_bytes=free_b,
                remote_sem=rsem.num,
                remote_sem_name=rsem.name,
                local_sem_update=bass.create_sync_update(lsem, 16),
                queue_num=0,
                dests=packed,
                relative=True,
            ))
        return nc.gpsimd._track_prepare_only(inst, 0)

    trig_k = trig_v = None
    k_preps = []
    slot_pe_readers = []   # PE matmuls reading krecv
    slot_dve_readers = []  # DVE ops reading vrecv

    with tile.TileContext(nc) as tc, nc.allow_low_precision(
            reason="bf16 tiles feed matmuls; fp32 accumulation in PSUM"):
        with (
            tc.tile_pool(name="persist", bufs=1) as persist,
            tc.tile_pool(name="post", bufs=1) as post,
            tc.tile_pool(name="w1pre", bufs=1) as w1pre_pool,
        ):
            # Constants / small inputs
            ones_f32 = persist.tile([128, 128], F32)
            nc.vector.memset(ones_f32[:, :], 1.0)
            ones_col = persist.tile([128, 1], BF16)
            nc.vector.tensor_copy(ones_col[:, :], ones_f32[:, 0:1])
            ones_bf = persist.tile([128, 8], BF16)
            nc.vector.tensor_copy(ones_bf[:, :], ones_f32[:, 0:8])
            ones_row = persist.tile([1, 128], F32R)
            nc.vector.tensor_copy(ones_row[:, :], ones_f32[0:1, :])
            eps_t = persist.tile([1, 1], F32)
            nc.vector.memset(eps_t[:, :], EPS)
            ebase = persist.tile([128, 512], F32)
            nc.vector.memset(ebase[:, :], EBASE)

            scal_sb = persist.tile([128, 176], F32)
            bq_sb = scal_sb[:, 0:8]
            bk_sb = scal_sb[:, 8:16]
            bv_sb = scal_sb[0:64, 16:32]
            bo_sb = scal_sb[:, 32:40]
            b1_sb = scal_sb[:, 40:72]
            b2_sb = scal_sb[:, 72:80]
            g1_sb = scal_sb[:, 80:88]
            bt1_sb = scal_sb[:, 88:96]
            g2_sb = scal_sb[:, 96:104]
            bt2_sb = scal_sb[:, 104:112]
            uneg_sb = scal_sb[:, 112:144]
            vb1_sb = scal_sb[:, 144:176]
            kvind_sb = persist.tile([128, 12, 8], F32)
            nc.gpsimd.dma_start(out=kvind_sb[:, :, :],
                                in_=kvind.rearrange("j p c -> p j c"))

            z1 = post.tile([128, 8, 512], BF16, tag="z")
            y1 = post.tile([128, 8, 512], BF16, tag="y")

            with (
                tc.tile_pool(name="span1", bufs=1) as span1,
                tc.tile_pool(name="kvbuf", bufs=1) as kvbuf,
                tc.tile_pool(name="wo_sb", bufs=2) as wo_pool,
            ):
                # Own-chunk x^T (feature-major), also the residual input.
                xq = span1.tile([128, 8, 512], BF16)
                for ci in range(CT):
                    nc.scalar.dma_start(
                        out=xq[:, ci, :],
                        in_=xc_d[128 * ci:128 * ci + 128, :])
                nc.gpsimd.dma_start(out=scal_sb[:, :], in_=scal[:, :])
                masks_sb = span1.tile([128, 5, 512], BF16)

                # K/V own + recv buffers (alive for all of attention)
                ks = kvbuf.tile([128, 8, 512], BF16)
                vs = kvbuf.tile([128, 4, 1024], BF16)
                krecv = kvbuf.tile([128, 3, 8, 512], BF16)
                vrecv = kvbuf.tile([128, 3, 4, 1024], BF16)

                # ---- K own: 8 feature slabs (head pairs) ----
                with (
                    tc.tile_pool(name="wkv", bufs=3) as wkv,
                    tc.tile_pool(name="kv_ps", bufs=2, space="PSUM") as kv_ps,
                ):
                    for sp in range(4):
                        wks = wkv.tile([128, 8, 256], BF16, tag="wks")
                        nc.sync.dma_start(
                            out=wks[:, :, :],
                            in_=wk[sp, :, :]
                            .rearrange("(a p) f -> p a f", p=128))
                        for half in range(2):
                            s = 2 * sp + half
                            kps = kv_ps.tile([128, 512], F32, tag="kvp")
                            for ci in range(CT):
                                nc.tensor.matmul(
                                    kps[:, :],
                                    wks[:, ci,
                                        128 * half:128 * half + 128],
                                    xq[:, ci, :],
                                    start=(ci == 0), stop=(ci == CT - 1))
                            nc.vector.tensor_scalar_add(
                                out=ks[:, s, :], in0=kps[:, :],
                                scalar1=bk_sb[:, s:s + 1])
                    # K broadcasts to the 3 quad peers (XOR-relative)
                    for d in (1, 2, 3):
                        k_preps.append(bcast4(krecv[:, d - 1, :, :],
                                              ks[:, :, :], d))
                    trig_k = nc.gpsimd.trigger_dma(count=None)

                    # ---- V own: 4 token tiles x 4 feature chunks ----
                    for fq in range(4):
                        wvs = wkv.tile([128, 8, 256], BF16, tag="wvs")
                        nc.sync.dma_start(
                            out=wvs[:, :, :],
                            in_=wv[:, 256 * fq:256 * fq + 256]
                            .rearrange("(a p) f -> p a f", p=128))
                        for tt in range(4):
                            vps = kv_ps.tile([128, 256], F32, tag="kvp")
                            for ci in range(CT):
                                nc.tensor.matmul(
                                    vps[:, :],
                                    xq[:, ci, 128 * tt:128 * tt + 128],
                                    wvs[:, ci, :],
                                    start=(ci == 0), stop=(ci == CT - 1))
                            nc.scalar.copy(
                                vs[:, tt, 256 * fq:256 * fq + 256],
                                vps[:, :])
                    for d in (1, 2, 3):
                        bcast4(vrecv[:, d - 1, :, :], vs[:, :, :], d)
                    trig_v = nc.gpsimd.trigger_dma(count=None)

                # MHA output, feature-major: head pair on partitions
                mha = span1.tile([128, 8, 512], BF16)

                # ------------- Attention: 4 passes of 4 heads -------------
                with (
                    tc.tile_pool(name="wq_sb", bufs=1) as wq_pool,
                    tc.tile_pool(name="attn_sb", bufs=4) as attn_sb,
                    tc.tile_pool(name="e_sb", bufs=8) as e_sb,
                    tc.tile_pool(name="stg_sb", bufs=4) as stg_sb,
                    tc.tile_pool(name="n_sb", bufs=4) as n_sb,
                ):
                    # full Wq upfront; all 4 passes' Q projections run
                    # before the peer gate so the kv-exchange barrier hides
                    # under them.
                    for mj in range(5):
                        nc.gpsimd.dma_start(
                            out=masks_sb[:, mj, :],
                            in_=masks[mj, :, :])
                    wq_sb = wq_pool.tile([128, 8, 1024], BF16, tag="wqf")
                    for qq in range(4):
                        nc.sync.dma_start(
                            out=wq_sb[:, 2 * qq:2 * qq + 2, :],
                            in_=wq[256 * qq:256 * qq + 256, :]
                            .rearrange("(a p) f -> p a f", p=128))
                    tc.tile_set_cur_wait(0.032)
                    w1pre = w1pre_pool.tile([128, 8, 512], BF16, name="w1s0")
                    qts = []
                    with tc.tile_pool(name="q_ps", bufs=2,
                                      space="PSUM") as q_ps:
                        for qr in range(4):
                            qt = attn_sb.tile([128, 2, 512], BF16, tag="qt",
                                              name=f"qt{qr}")
                            for kd in range(2):
                                qps = q_ps.tile([128, 512], F32, tag="qp")
                                for ci in range(CT):
                                    nc.tensor.matmul(
                                        qps[:, :],
                                        wq_sb[:, ci,
                                              256 * qr + 128 * kd:
                                              256 * qr + 128 * kd + 128],
                                        xq[:, ci, :],
                                        start=(ci == 0), stop=(ci == CT - 1))
                                nc.vector.tensor_scalar_add(
                                    out=qt[:, kd, :], in0=qps[:, :],
                                    scalar1=bq_sb[:, 2 * qr + kd:
                                                  2 * qr + kd + 1])
                            qts.append(qt)

                    ident = masks_sb[:, 4, 0:128]
                    # greedy build-time load balance of exp tiles between
                    # the Act engine (direct from PSUM) and GpSimd pow
                    # (staged PSUM->SBUF by DMA: GPSIMD cannot read PSUM;
                    # the DMA engines are idle during attention).
                    exp_t = [0.0, 0.0]

                    def emit_exp(e_ap, l_ap, stage_pool, bias_act=None,
                                 bias_pool=None, force_act=False):
                        """Visibility of whole peer kv-tiles is folded into
                        the exp as a large negative per-partition bias
                        (bias_act post-scale for Act, bias_pool pre-scale
                        added during the PSUM->SBUF staging copy)."""
                        if force_act or exp_t[0] + 612 <= exp_t[1] + 1167:
                            exp_t[0] += 612
                            nc.scalar.activation(
                                e_ap, l_ap, AF.Exp, scale=SCALE,
                                bias=bias_act if bias_act is not None
                                else 0.0)
                        else:
                            exp_t[1] += 1167
                            stg = stage_pool.tile([128, 512], F32,
                                                  tag="pstg")
                            if bias_pool is None:
                                nc.vector.tensor_copy(stg[:, :], l_ap)
                            else:
                                nc.vector.tensor_scalar(
                                    out=stg[:, :], in0=l_ap,
                                    scalar1=bias_pool, scalar2=None,
                                    op0=ALU.add)
                            nc.gpsimd.tensor_tensor(e_ap, ebase[:, :],
                                                    stg[:, :], ALU.pow)

                    with (
                        tc.tile_pool(name="l_ps", bufs=4,
                                     space="PSUM") as l_ps,
                        tc.tile_pool(name="o_ps", bufs=1,
                                     space="PSUM") as o_ps,
                        tc.tile_pool(name="t_ps", bufs=1,
                                     space="PSUM") as t_ps,
                    ):
                        def emit_normalize(qr_p, oacc_p, dacc_p):
                            # per-query denominator is a per-partition
                            # scalar; PE-transpose two heads side by side
                            # back to feature-major mha
                            den = n_sb.tile([128, 16], F32, tag="den")
                            nc.vector.reciprocal(den[:, :], dacc_p[:, :])
                            for qi in range(4):
                                for hp in range(2):
                                    avp = n_sb.tile([128, 128], BF16,
                                                    tag="av")
                                    for sub in range(2):
                                        g = 2 * hp + sub
                                        nc.vector.tensor_scalar(
                                            out=avp[:, 64 * sub:
                                                    64 * sub + 64],
                                            in0=oacc_p[hp][:, 4 * sub + qi,
                                                           :],
                                            scalar1=den[:, 4 * g + qi:
                                                        4 * g + qi + 1],
                                            scalar2=None, op0=ALU.mult)
                                    tp = t_ps.tile([128, 1024], BF16,
                                                   tag="tp")
                                    nc.tensor.transpose(
                                        tp[:, 0:128], avp[:, :], ident)
                                    if hp == 0:
                                        nc.vector.tensor_copy(
                                            out=mha[:, 2 * qr_p + hp,
                                                    128 * qi:128 * qi + 128],
                                            in_=tp[:, 0:128])
                                    else:
                                        nc.scalar.copy(
                                            mha[:, 2 * qr_p + hp,
                                                128 * qi:128 * qi + 128],
                                            tp[:, 0:128])

                        pending = None
                        for qr in range(4):
                            qt = qts[qr]
                            # AV accumulators: head pair x 4 query tiles x
                            # 64 feats (query-major), one PSUM bank each;
                            # softmax denominators accumulate separately in
                            # a shared [128, 16] bank.
                            # One PSUM zero-region (2KB bank) per tile:
                            # exactly ONE start=True per region (first
                            # matmul), first-touch writes of other slots
                            # overwrite via pending-zero, one stop=True on
                            # the region's last matmul.
                            oacc = [o_ps.tile([128, 8, 64], F32,
                                              tag=f"o{hp}",
                                              name=f"o{hp}_{qr}")
                                    for hp in range(2)]
                            dacc_pad = o_ps.tile([128, 512], F32,
                                                 tag="dacc",
                                                 name=f"dacc_{qr}")
                            dacc = dacc_pad[:, 0:16]

                            def _av_part(es, vsrc, start, stop, record):
                                for g in range(4):
                                    for qi in range(4):
                                        first = g % 2 == 0 and qi == 0
                                        last = g % 2 == 1 and qi == 3
                                        mm2 = nc.tensor.matmul(
                                            oacc[g // 2][:, 4 * (g % 2) + qi,
                                                         :],
                                            es[g][:, 128 * qi:128 * qi + 128],
                                            vsrc(g),
                                            start=start and first,
                                            stop=stop and last)
                                        if record and g == 0 and qi == 0:
                                            slot_pe_readers.append(mm2.ins)
                                        nc.tensor.matmul(
                                            dacc[:, 4 * g + qi:4 * g + qi + 1],
                                            es[g][:, 128 * qi:128 * qi + 128],
                                            ones_col[:, :],
                                            start=start and g == 0 and qi == 0,
                                            stop=stop and g == 3 and qi == 3)

                            def _tile_pass(ksrc, vsrc, mask_mj, bias_j,
                                           start, stop, record=False,
                                           emit_av=True, force_act=False):
                                """One 128-token kv tile: logits+exp+AV.

                                ksrc: (lh, p) -> lhsT AP [64, 128]
                                vsrc: g -> AP [128, 64] (head g V tile)
                                mask_mj: diag mask index or None
                                bias_j: peer-slot index for visibility
                                bias columns, or None (diag, visible)
                                emit_av=False: return a thunk emitting
                                the AV matmuls later (pass-boundary
                                overlap: logits+exp of the next pass run
                                while the previous pass normalizes).
                                """
                                es = []
                                for p in range(2):
                                    for lh in range(2):
                                        lps = l_ps.tile([128, 512], F32,
                                                        tag="lg")
                                        mm = nc.tensor.matmul(
                                            lps[:, :], ksrc(lh, p),
                                            qt[64 * lh:64 * lh + 64, p, :],
                                            start=True, stop=True,
                                            tile_position=(64 * lh, 0))
                                        if record:
                                            slot_pe_readers.append(mm.ins)
                                        e = e_sb.tile([128, 512], BF16,
                                                      tag="e",
                                                      name=f"e{p}{lh}")
                                        if bias_j is None:
                                            emit_exp(e[:, :], lps[:, :],
                                                     stg_sb,
                                                     force_act=force_act)
                                        else:
                                            emit_exp(
                                                e[:, :], lps[:, :], stg_sb,
                                                bias_act=kvind_sb[
                                                    :, bias_j, 4:5],
                                                bias_pool=kvind_sb[
                                                    :, bias_j, 5:6],
                                                force_act=force_act)
                                        if mask_mj is not None:
                                            nc.vector.tensor_mul(
                                                e[:, :], e[:, :],
                                                masks_sb[:, mask_mj, :])
                                        es.append(e)
                                if emit_av:
                                    _av_part(es, vsrc, start, stop, record)
                                    return None
                                return lambda: _av_part(es, vsrc, start,
                                                        stop, record)

                            def _diag_args(tt):
                                return (
                                    lambda lh, p: ks[64 * lh:64 * lh + 64,
                                                     2 * qr + p,
                                                     128 * tt:128 * tt + 128],
                                    lambda g: vs[
                                        :, tt,
                                        256 * qr + 64 * g:
                                        256 * qr + 64 * g + 64],
                                    tt, None)

                            # first two diagonal tiles: logits+exp only,
                            # so Act/Pool stay fed while the previous
                            # pass's normalize runs on DVE/PE
                            av0 = _tile_pass(*_diag_args(0), start=True,
                                             stop=False, emit_av=False,
                                             force_act=False)
                            av1 = _tile_pass(*_diag_args(1), start=False,
                                             stop=False, emit_av=False,
                                             force_act=False)
                            if pending is not None:
                                emit_normalize(*pending)
                            av0()
                            av1()
                            for tt in range(2, 4):
                                _tile_pass(*_diag_args(tt), start=False,
                                           stop=False)
                            # peer-slot tiles, from krecv/vrecv
                            for j in range(12):
                                d, tt = divmod(j, 4)
                                _tile_pass(
                                    lambda lh, p: krecv[
                                        64 * lh:64 * lh + 64, d,
                                        2 * qr + p,
                                        128 * tt:128 * tt + 128],
                                    lambda g, d=d, tt=tt: vrecv[
                                        :, d, tt,
                                        256 * qr + 64 * g:
                                        256 * qr + 64 * g + 64],
                                    None, j,
                                    start=False, stop=(j == 11), record=True)
                            pending = (qr, oacc, dacc)

                            if qr == 1:
                                # w1 stripe 0 (consumed at FFN1, far away)
                                for hf_ in range(4):
                                    nc.scalar.dma_start(
                                        out=w1pre[:, 2 * hf_:2 * hf_ + 2, :],
                                        in_=w1[256 * hf_:256 * hf_ + 256,
                                               0:512]
                                        .rearrange("(a p) f -> p a f", p=128))
                            if qr == 2:
                                # prefetch WO while pass 3 computes
                                wosbs = []
                                for gh in range(2):
                                    wosb = wo_pool.tile(
                                        [128, 4, C], BF16, tag="wosb",
                                        name=f"wosb{gh}")
                                    for hf in range(2):
                                        nc.sync.dma_start(
                                            out=wosb[:, 2 * hf:2 * hf + 2, :],
                                            in_=wo[4 * gh + 2 * hf:
                                                   4 * gh + 2 * hf + 2, :, :]
                                            .rearrange("h p f -> p h f"))
                                    wosbs.append(wosb)

                        emit_normalize(*pending)

                if dbg is not None:
                    for sl in range(8):
                        nc.sync.dma_start(out=dbg[:, sl, :],
                                          in_=mha[:, sl, :])
                tc.tile_set_cur_wait(0, enable=True)
                # ---- WO + residual -> Z1, LN1 stats interleaved ----
                with (
                    tc.tile_pool(name="wo_ps", bufs=3, space="PSUM") as wo_ps,
                    tc.tile_pool(name="st1_ps", bufs=1,
                                 space="PSUM") as st1_ps,
                    tc.tile_pool(name="st1_sb", bufs=2) as st1_sb,
                ):
                    m1_ps = st1_ps.tile([1, 512], F32, tag="ln_m")
                    sq1_ps = st1_ps.tile([1, 512], F32, tag="ln_sq")
                    for co in range(CT):
                        wop = wo_ps.tile([128, 512], F32, tag="wop")
                        for p in range(8):
                            nc.tensor.matmul(
                                wop[:, :],
                                wosbs[p // 4][:, p % 4,
                                              128 * co:128 * co + 128],
                                mha[:, p, :],
                                start=(p == 0), stop=(p == 7))
                        nc.vector.scalar_tensor_tensor(
                            out=z1[:, co, :], in0=wop[:, :],
                            scalar=bo_sb[:, co:co + 1], in1=xq[:, co, :],
                            op0=ALU.add, op1=ALU.add)
                        _ln_stats_step(nc, st1_ps, st1_sb, m1_ps, sq1_ps,
                                       z1[:, co, :], ones_col, co,
                                       sq_eng=nc.vector)

            # ------------- LN1 -> y1 (FFN weight prefetch overlaps) ----
            z2 = post.tile([128, 8, 512], BF16, tag="z")
            with (
                tc.tile_pool(name="ffn_h", bufs=1) as ffn_h,
                tc.tile_pool(name="w1_sb", bufs=4) as w1_pool,
                tc.tile_pool(name="w2_sb", bufs=2) as w2_pool,
            ):
                def _load_w1s(s, pool=None):
                    t = (pool or w1_pool).tile(
                        [128, 8, 512], BF16, tag="w1s", name=f"w1s{s}")
                    for hf in range(4):
                        nc.sync.dma_start(
                            out=t[:, 2 * hf:2 * hf + 2, :],
                            in_=w1[256 * hf:256 * hf + 256,
                                   512 * s:512 * s + 512]
                            .rearrange("(a p) f -> p a f", p=128))
                    return t

                def _load_w2c(co):
                    t = w2_pool.tile([128, FFT, 128], BF16, tag="w2c",
                                     name=f"w2c{co}")
                    for hf in range(2):
                        nc.sync.dma_start(
                            out=t[:, 16 * hf:16 * hf + 16, :],
                            in_=w2[2048 * hf:2048 * hf + 2048,
                                   128 * co:128 * co + 128]
                            .rearrange("(a p) n -> p a n", p=128))
                    return t

                w1s_next = w1pre
                w2c_next = _load_w2c(0)
                with (
                    tc.tile_pool(name="stat_ps1", bufs=1,
                                 space="PSUM") as stat_ps,
                    tc.tile_pool(name="stat_sb1", bufs=4) as stat_sb,
                    tc.tile_pool(name="hfix_sb", bufs=6) as hfix_sb,
                ):
                    bcm1, bcr1 = _layernorm_feature_major(
                        nc, tc, persist, stat_ps, stat_sb,
                        lambda c: z1[:, c, :], y1, g1_sb, bt1_sb,
                        ones_col, ones_row, eps_t, stats=(m1_ps, sq1_ps))

                    hbuf = ffn_h.tile([128, FFT, 512], BF16)
                    # pass 1 on UNNORMALIZED z1 (gamma folded into W1 on
                    # host): h = relu(r*(h_pre - m*u) + v + b1) where
                    # u = sum_f W1g[f,:], v = sum_f W1[f,:]*beta1[f]
                    with tc.tile_pool(name="h_ps", bufs=6,
                                      space="PSUM") as h_ps:
                        for s in range(8):  # 8 stripes of 512 ff cols
                            w1s = w1s_next
                            if s < 7:
                                w1s_next = _load_w1s(s + 1)
                            for k in range(4):
                                f = 4 * s + k
                                hps = h_ps.tile([128, 512], F32, tag="hps")
                                for ci in range(CT):
                                    nc.tensor.matmul(
                                        hps[:, :],
                                        w1s[:, ci, 128 * k:128 * k + 128],
                                        z1[:, ci, :],
                                        start=(ci == 0), stop=(ci == CT - 1))
                                t1 = hfix_sb.tile([128, 512], F32,
                                                  tag="t1")
                                nc.vector.scalar_tensor_tensor(
                                    out=t1[:, :], in0=bcm1[:, :],
                                    scalar=uneg_sb[:, f:f + 1],
                                    in1=hps[:, :],
                                    op0=ALU.mult, op1=ALU.add)
                                t2 = hfix_sb.tile([128, 512], F32,
                                                  tag="t2")
                                nc.gpsimd.tensor_mul(t2[:, :], t1[:, :],
                                                     bcr1[:, :])
                                nc.scalar.activation(
                                    hbuf[:, f, :], t2[:, :], AF.Relu,
                                    bias=vb1_sb[:, f:f + 1])
                # pass 2: z2 = h @ W2 + b2 + y1, output-column major,
                # LN2 stats accumulated as each column lands
                with (
                    tc.tile_pool(name="o2_ps", bufs=3,
                                 space="PSUM") as o2_ps,
                    tc.tile_pool(name="st2_ps", bufs=1,
                                 space="PSUM") as st2_ps,
                    tc.tile_pool(name="st2_sb", bufs=2) as st2_sb,
                ):
                    m2_ps = st2_ps.tile([1, 512], F32, tag="ln_m")
                    sq2_ps = st2_ps.tile([1, 512], F32, tag="ln_sq")
                    for co in range(CT):
                        w2c = w2c_next
                        if co < CT - 1:
                            w2c_next = _load_w2c(co + 1)
                        o2t = o2_ps.tile([128, 512], F32, tag="o2")
                        for f in range(FFT):
                            nc.tensor.matmul(
                                o2t[:, :],
                                w2c[:, f, :],
                                hbuf[:, f, :],
                                start=(f == 0), stop=(f == FFT - 1))
                        nc.vector.scalar_tensor_tensor(
                            out=z2[:, co, :], in0=o2t[:, :],
                            scalar=b2_sb[:, co:co + 1], in1=y1[:, co, :],
                            op0=ALU.add, op1=ALU.add)
                        _ln_stats_step(nc, st2_ps, st2_sb, m2_ps, sq2_ps,
                                       z2[:, co, :], ones_col, co,
                                       sq_eng=nc.vector)

            # ------------- LN2 -> output -------------
            with (
                tc.tile_pool(name="stat_ps2", bufs=1, space="PSUM") as stat_ps2,
                tc.tile_pool(name="stat_sb2", bufs=6) as stat_sb2,
            ):
                y2 = post.tile([128, 8, 512], F32, tag="y")

                def _out_dma(c):
                    nc.sync.dma_start(out=out[128 * c:128 * c + 128, :],
                                      in_=y2[:, c, :])

                _layernorm_feature_major(
                    nc, tc, persist, stat_ps2, stat_sb2,
                    lambda c: z2[:, c, :], y2, g2_sb, bt2_sb,
                    ones_col, ones_row, eps_t, out_dma=_out_dma,
                    stats=(m2_ps, sq2_ps))

    # ---- post-schedule insertion of the RDMA sync protocol ----
    fn = nc.m.functions[0]

    def find_block(ins):
        for b in fn.blocks:
            if any(i is ins for i in b.instructions):
                return b
        raise KeyError(ins.name)

    def insert(ins_list, anchor, after):
        b = find_block(anchor)
        for x in ins_list:
            bb = find_block(x)
            bb.instructions.remove(x)
        idx = next(i for i, v in enumerate(b.instructions) if v is anchor)
        if after:
            idx += 1
        for x in reversed(ins_list):
            b.instructions.insert(idx, x)

    # after each trigger: drain sends (HW: blocks until DMA queues are
    # empty), then a quad barrier collective whose then_inc gates readers
    def kv_barrier(tag, trig_ins, sem):
        drn = nc.gpsimd.drain()
        b_in = nc.dram_tensor(f"kvbar_{tag}_in", [1, 1], mybir.dt.uint8)
        b_out = nc.dram_tensor(f"kvbar_{tag}_out", [4, 1], mybir.dt.uint8)
        cc = nc.gpsimd.collective_compute(
            "AllGather", ALU.bypass, RG, [b_in.ap()], [b_out.ap()])
        bass.BassInstruction(cc.ins).then_inc(sem, 1)
        insert([drn.ins, cc.ins], trig_ins, after=True)

    kv_barrier("kv", trig_v.ins, arrk_sem)

    # entry barrier: no RDMA packet may land before every quad peer has
    # entered this execution (protects recv buffers across runs); placed
    # right before the first send prep so kernel start doesn't stall on it
    ebw = nc.gpsimd.bir_kernel_barrier_wait(RG)
    cl1 = nc.gpsimd.sem_clear(arrk_sem)
    cl2 = nc.gpsimd.sem_clear(arrv_sem)
    cl3 = nc.gpsimd.sem_clear(rsem)
    cl4 = nc.gpsimd.sem_clear(lsem)
    insert([ebw.ins, cl1.ins, cl2.ins, cl3.ins, cl4.ins],
           k_preps[0].ins, after=False)

    # reader gates: PE before first krecv matmul, DVE before first
    # vrecv copy (min block index over all recorded readers)
    blk = find_block(slot_pe_readers[0])
    order = {id(v): i for i, v in enumerate(blk.instructions)}
    first_pe = min(slot_pe_readers, key=lambda i: order[id(i)])
    w_pe = nc.tensor.wait_ge(arrk_sem, 1)
    insert([w_pe.ins], first_pe, after=False)
    if slot_dve_readers:
        first_dve = min(slot_dve_readers, key=lambda i: order[id(i)])
        w_dve = nc.vector.wait_ge(arrk_sem, 1)
        insert([w_dve.ins], first_dve, after=False)

    nc.compile()
    return nc


def _prep_inputs(x, Wqkv, bqkv, WO, bO, gamma1, beta1, gamma2, beta2,
                 W1, b1, W2, b2):
    """Build the 8 per-core input maps (all host-side numpy)."""
    f32 = np.float32
    bf16 = BF16_NP
    x = np.asarray(x, f32)
    Wqkv = np.asarray(Wqkv, f32)
    bqkv = np.asarray(bqkv, f32)

    # head-major feature-ordered projection weights [C, 1024]
    wq_np = np.ascontiguousarray(
        Wqkv[:, :, 0:DK].transpose(1, 0, 2).reshape(C, C).astype(bf16))
    wk_np = np.ascontiguousarray(
        Wqkv[:, :, DK:2 * DK].transpose(1, 0, 2).reshape(C, C)
        .reshape(C, 4, 256).transpose(1, 0, 2).astype(bf16))
    wv_np = np.ascontiguousarray(
        Wqkv[:, :, 2 * DK:3 * DK].transpose(1, 0, 2).reshape(C, C).astype(bf16))
    wo_np = np.ascontiguousarray(np.asarray(WO, f32).reshape(8, 128, C)
                                 .astype(bf16))
    w1g = np.asarray(W1, f32) * np.asarray(gamma1, f32)[:, None]
    w1_np = np.ascontiguousarray(w1g.astype(bf16))
    u_np = w1_np.astype(f32).sum(axis=0)          # sum_f W1g[f, :]
    v_np = (np.asarray(W1, f32)
            * np.asarray(beta1, f32)[:, None]).sum(axis=0)
    w2_np = np.ascontiguousarray(np.asarray(W2, f32).astype(bf16))

    def col8(v):  # [1024] -> [128, 8] (col j = elements 128j:128j+128)
        return np.ascontiguousarray(np.asarray(v, f32).reshape(8, 128).T)

    # V bias folded into the WO bias: attn weights sum to 1, so
    # mha = raw_av + bv  =>  mha@WO + bO == raw_av@WO + (bO + bv@WO).
    bv_full = bqkv[:, 2 * DK:3 * DK].reshape(C).astype(f32)
    bo_eff = np.asarray(bO, f32) + bv_full @ np.asarray(WO, f32)

    scal_np = np.zeros((128, 176), f32)
    scal_np[:, 0:8] = col8(bqkv[:, 0:DK].reshape(C))
    scal_np[:, 8:16] = col8(bqkv[:, DK:2 * DK].reshape(C))
    scal_np[:, 32:40] = col8(bo_eff)
    scal_np[:, 40:72] = np.asarray(b1, f32).reshape(32, 128).T
    scal_np[:, 72:80] = col8(b2)
    scal_np[:, 80:88] = col8(gamma1)
    scal_np[:, 88:96] = col8(beta1)
    scal_np[:, 96:104] = col8(gamma2)
    scal_np[:, 104:112] = col8(beta2)
    scal_np[:, 112:144] = (-u_np).reshape(32, 128).T
    scal_np[:, 144:176] = (v_np + np.asarray(b1, f32)).reshape(32, 128).T
    scal_np = np.ascontiguousarray(scal_np)

    # causal masks for the 4 own-chunk diagonal tiles (same on every core)
    # + identity matrix for PE transposes in slot 4
    tq = np.arange(512)[None, :]
    masks_np = np.zeros((5, 128, 512), bf16)
    for j in range(4):
        tk = (128 * j + np.arange(128))[:, None]
        masks_np[j] = (tq >= tk).astype(bf16)
    masks_np[4, :, 0:128] = np.eye(128, dtype=bf16)

    in_maps = []
    for r in range(NCORES):
        b, ch = divmod(r, 4)
        qs = QCH * ch
        xc_np = np.ascontiguousarray(x[b].T[:, qs:qs + QCH].astype(bf16))
        # peer-slot visibility: slot d-1 holds chunk (ch ^ d)
        kvind_np = np.zeros((12, 128, 8), f32)
        for d in (1, 2, 3):
            vis = 1.0 if (ch ^ d) < ch else 0.0
            kvind_np[4 * (d - 1):4 * d, :, 0:4] = vis
            # visibility as a large negative exp bias: col 4 post-scale
            # (Act Exp bias), col 5 pre-scale (added before gpsimd pow)
            kvind_np[4 * (d - 1):4 * d, :, 4] = (vis - 1.0) * 38.0
            kvind_np[4 * (d - 1):4 * d, :, 5] = (vis - 1.0) * 304.0
        in_maps.append({
            "xc": xc_np,
            "wq": wq_np, "wk": wk_np, "wv": wv_np, "wo": wo_np,
            "w1": w1_np, "w2": w2_np,
            "masks": masks_np, "kvind": np.ascontiguousarray(kvind_np),
            "scal": scal_np,
        })
    return in_maps


def kernel(**inputs):
    if "nc" not in _CACHE:
        _CACHE["nc"] = _build()
    nc = _CACHE["nc"]
    in_maps = _prep_inputs(**inputs)
    trace = os.environ.get("KERNEL_TRACE", "0") == "1"
    res = run_bass_kernel_spmd(nc, in_maps, core_ids=list(range(NCORES)),
                               trace=trace)
    _CACHE["last_result"] = res
    out = np.empty((B, T, C), np.float32)
    for r in range(NCORES):
        b, ch = divmod(r, 4)
        out[b, QCH * ch:QCH * ch + QCH, :] = res.results[r]["out"].T
    return out

